# revision 1
# baseline (speedup 1.0000x reference)
"""Blake2 soft-cipher Bass kernel for Trainium2 (8 NeuronCores, data parallel)."""
import sys
sys.path.insert(0, "/opt/trn_rl_repo")
import math
import os
import numpy as np
from concourse import bass, mybir
from concourse.tile import TileContext
from concourse.bass_primitives_rust import SemaphoreHandle
from concourse import bass_primitives_rust as _bpr
from concourse.bass import _bass_rust

A = mybir.AluOpType
F = mybir.ActivationFunctionType
DT = mybir.dt.float32

# ---------------------------------------------------------------- geometry
P = 128
FD = 980
BLOCK_ROWS = P * FD
BLOCKS = 2
CORE_ROWS = BLOCK_ROWS * BLOCKS
N_CORES = 8
TOTAL_ROWS = 2_000_000
PAD_ROWS = CORE_ROWS * N_CORES

ROUNDS = 10
G_SCHEDULE = [
    (0, 4, 8, 12, 0, 1), (1, 5, 9, 13, 2, 3), (2, 6, 10, 14, 4, 5), (3, 7, 11, 15, 6, 7),
    (0, 5, 10, 15, 8, 9), (1, 6, 11, 12, 10, 11), (2, 7, 8, 13, 12, 13), (3, 4, 9, 14, 14, 15),
]
_IV_INTS = [7640891576956012808, 13503953896175478587, 4354685564936845355,
            11912009170470909681, 5840696475078001361, 11170449401992604703,
            2270897969802886507, 6620516959819538809]
IV = (np.asarray(_IV_INTS, dtype=np.float32) / np.float32(2.0**64)).astype(np.float32)
STEEP = np.float32(10.0)


def f32(x):
    return np.float32(x)


def sig_const(z):
    return f32(1.0 / (1.0 + math.exp(-float(z))))


def configure(fd=980, blocks=2):
    global FD, BLOCK_ROWS, BLOCKS, CORE_ROWS, PAD_ROWS
    FD = fd
    BLOCKS = blocks
    BLOCK_ROWS = P * FD
    CORE_ROWS = BLOCK_ROWS * BLOCKS
    PAD_ROWS = CORE_ROWS * N_CORES


class Val:
    def __init__(self, const=None, ap=None, scale=None):
        self.const = const
        self.ap = ap          # () -> AP
        self.scale = scale    # pending multiply-by-2^-n (rot16/24 folding)

    @property
    def is_const(self):
        return self.const is not None


def ns_tt():
    return (FD + 58) / 0.96

def ns_ts():
    return (FD / 2 + 58) / 0.96

def ns_act():
    return (FD + 352) / 1.2


class Program2:
    def __init__(self):
        self.nc = bass.Bass("TRN2")
        self.est = {"dve": 0.0, "act": 0.0}
        self._lane = None
        self._lane_id = 0

    def _run(self, fn):
        fn()

    def begin_lane(self, lane, lane_id=0):
        self._lane = lane
        self._lane_id = lane_id

    def end_lane(self):
        self._lane = None

    def merge_lanes(self, lanes):
        lanes = [list(l) for l in lanes if l]
        while lanes:
            nxt = []
            for l in lanes:
                l.pop(0)()
                if l:
                    nxt.append(l)
            lanes = nxt

    # ---------- low-level emitters (inside TileContext)
    def dve_tt(self, out, a, b, op):
        self._run(lambda: self.nc.vector.tensor_tensor(out(), a(), b(), op=op))
        self.est["dve"] += ns_tt()

    def dve_stt(self, out, in0, scalar, in1, op0, op1, rev0=False, rev1=False):
        def f():
            i = self.nc.vector.scalar_tensor_tensor(out(), in0(), scalar, in1(), op0=op0, op1=op1)
            if rev0:
                i.ins.reverse0 = True
            if rev1:
                i.ins.reverse1 = True
        self._run(f)
        self.est["dve"] += ns_tt()

    def dve_ts(self, out, in0, s1, s2, op0, op1=None, rev0=False):
        def f():
            if op1 is None:
                i = self.nc.vector.tensor_scalar(out(), in0(), s1, None, op0=op0)
            else:
                i = self.nc.vector.tensor_scalar(out(), in0(), s1, s2, op0=op0, op1=op1)
            if rev0:
                i.ins.reverse0 = True
        self._run(f)
        self.est["dve"] += ns_ts()

    def act_act(self, out, in0, func, bias, scale, bias_ap=None):
        def f():
            b = bias_ap() if bias_ap is not None else bias
            self.nc.scalar.activation(out(), in0(), func, bias=b, scale=scale)
        self._run(f)
        self.est["act"] += ns_act()

    def affine(self, out, in0, scale, bias):
        if self.est["dve"] * 0.98 > self.est["act"]:
            self.act_act(out, in0, F.Copy, float(bias), float(scale))
        else:
            if bias == 0.0:
                self.dve_ts(out, in0, float(scale), None, A.mult)
            else:
                self.dve_ts(out, in0, float(scale), float(bias), A.mult, A.add)

    def affine1m(self, out, in0):
        if self.est["dve"] * 0.98 > self.est["act"]:
            self.act_act(out, in0, F.Copy, 1.0, -1.0)
        else:
            self.dve_ts(out, in0, 1.0, None, A.subtract, rev0=True)

    # ---------- scratch (lazy; per-lane tag so slot rotation stays in-lane)
    def scr(self):
        cell = {}
        tag = f"scr{self._lane_id or 0}"
        def get():
            if "t" not in cell:
                cell["t"] = self.scr_pool.tile([P, FD], DT, tag=tag, name=tag, bufs=3)
            return cell["t"][:]
        return get

    # ---------- math primitives
    def sigmoid_act(self, out, in_ap, which, in_scale=None):
        bb = self.bias_m10_ap if which == "add" else self.bias_m5_ap
        sc = float(STEEP) * float(in_scale) if in_scale is not None else float(STEEP)
        self.act_act(out, in_ap, F.Sigmoid, None, sc, bias_ap=bb)

    def soft_add(self, dst_slot, aval, bval, dst_ap=None):
        if aval.is_const and bval.is_const:
            s = f32(aval.const + bval.const)
            wrap = sig_const(STEEP * (s - f32(1.0)))
            return Val(const=f32(s - wrap))
        dst = dst_ap if dst_ap is not None else self.v_aps[dst_slot]
        if aval.is_const or bval.is_const:
            c = aval.const if aval.is_const else bval.const
            tv = bval if aval.is_const else aval
            if tv.scale is not None:
                self.dve_ts(dst, tv.ap, float(tv.scale), float(c), A.mult, A.add)
            else:
                self.dve_ts(dst, tv.ap, float(c), None, A.add)
        elif aval.scale is not None or bval.scale is not None:
            sv, ov = (aval, bval) if aval.scale is not None else (bval, aval)
            assert ov.scale is None
            self.dve_stt(dst, sv.ap, float(sv.scale), ov.ap, op0=A.mult, op1=A.add)
        else:
            self.dve_tt(dst, aval.ap, bval.ap, A.add)
        w = self.scr()
        self.sigmoid_act(w, dst, "add")
        self.dve_tt(dst, dst, w, A.subtract)
        return Val(ap=dst)

    def soft_xor(self, dst_slot, xval, yval, dst_ap=None):
        if xval.is_const and yval.is_const:
            xs = sig_const(STEEP * (xval.const - f32(0.5)))
            ys = sig_const(STEEP * (yval.const - f32(0.5)))
            t1 = f32(xs * f32(1.0 - ys)); t2 = f32(f32(1.0 - xs) * ys)
            r = f32(f32(t1 + t2) - f32(t1 * t2))
            return Val(const=f32(min(max(r, 0.0), 1.0)))
        dst = dst_ap if dst_ap is not None else self.v_aps[dst_slot]
        if xval.is_const or yval.is_const:
            c = xval.const if xval.is_const else yval.const
            t = yval.ap if xval.is_const else xval.ap
            xs_c = sig_const(STEEP * (c - f32(0.5)))
            tsc = (yval if xval.is_const else xval).scale
            ys = self.scr(); s_b = self.scr(); t1b = self.scr()
            self.sigmoid_act(ys, t, "xor", in_scale=tsc)
            self.affine(s_b, ys, f32(1.0 - 2.0 * float(xs_c)), xs_c)
            self.affine(t1b, ys, f32(-float(xs_c)), xs_c)
            self.affine(ys, ys, f32(1.0 - float(xs_c)), f32(0.0))
            self.dve_tt(t1b, t1b, ys, A.mult)
            self.dve_tt(dst, s_b, t1b, A.subtract)
            return Val(ap=dst)
        xs = self.scr(); ys = self.scr(); t1 = self.scr()
        self.sigmoid_act(xs, xval.ap, "xor", in_scale=xval.scale)
        self.sigmoid_act(ys, yval.ap, "xor", in_scale=yval.scale)
        self.dve_stt(t1, ys, 1.0, xs, op0=A.subtract, op1=A.mult, rev0=True)
        self.dve_stt(xs, xs, 1.0, ys, op0=A.subtract, op1=A.mult, rev0=True)
        self.affine1m(ys, xs)
        self.dve_stt(t1, t1, 1.0, ys, op0=A.subtract, op1=A.mult, rev0=True)
        self.affine1m(dst, t1)
        return Val(ap=dst)

    def rotate(self, slot, n, val):
        assert not val.is_const and val.scale is None
        if n in (16, 24):
            # soft_xor output >= ~8.9e-5 > 2^(23-K): the wrapped fraction is
            # identically zero, so the rotate is an exact scale by 2^-n.
            # Defer it into the consumers (soft_add STT / sigmoid scale).
            return Val(ap=val.ap, scale=f32(2.0 ** (-n)))
        if n == 63:
            # x in [0,1): frac(2x) = 2x - [x>=0.5]; result + x*2^-63 < 1 always,
            # so no second frac is needed.
            x = val.ap
            dst = self.v_aps[slot]
            mask = self.scr(); fb = self.scr()
            self.dve_ts(mask, x, 0.5, None, A.is_ge)
            self.dve_stt(fb, x, 2.0, mask, op0=A.mult, op1=A.subtract)
            self.dve_stt(dst, x, float(2.0 ** -63), fb, op0=A.mult, op1=A.add)
            return Val(ap=dst)
        M23 = float(2.0 ** 23)
        M15 = float(3.0 * 2.0 ** 22)
        x = val.ap
        dst = self.v_aps[slot]
        c = self.scr(); u = self.scr()
        self.dve_ts(c, x, float(2.0 ** (64 - n)), M23, A.mult, A.min)
        self.affine(u, c, f32(1.0), f32(M23))
        self.dve_stt(u, u, M23, c, op0=A.subtract, op1=A.add, rev0=True)
        self.dve_stt(c, x, float(2.0 ** (-n)), u, op0=A.mult, op1=A.add)
        self.dve_ts(u, c, 0.5, M15, A.subtract, A.add)
        self.dve_stt(dst, u, M15, c, op0=A.subtract, op1=A.add, rev0=True)
        return Val(ap=dst)

    def G(self, vals, a, b, c, d, xi, yi):
        mx = Val(ap=self.m_aps[xi])
        my = Val(ap=self.m_aps[yi])
        vals[a] = self.soft_add(a, vals[a], vals[b])
        vals[a] = self.soft_add(a, vals[a], mx)
        vals[d] = self.soft_xor(d, vals[d], vals[a])
        vals[d] = self.rotate(d, 32, vals[d])
        vals[c] = self.soft_add(c, vals[c], vals[d])
        vals[b] = self.soft_xor(b, vals[b], vals[c])
        vals[b] = self.rotate(b, 24, vals[b])
        vals[a] = self.soft_add(a, vals[a], vals[b])
        vals[a] = self.soft_add(a, vals[a], my)
        vals[d] = self.soft_xor(d, vals[d], vals[a])
        vals[d] = self.rotate(d, 16, vals[d])
        vals[c] = self.soft_add(c, vals[c], vals[d])
        vals[b] = self.soft_xor(b, vals[b], vals[c])
        vals[b] = self.rotate(b, 63, vals[b])

    # ---------- whole program
    def build(self, scr_bufs=12):
        nc = self.nc
        self.msg = nc.declare_dram_parameter("message", [CORE_ROWS, 16], DT, isOutput=False)
        self.out = nc.declare_dram_parameter("out", [CORE_ROWS, 8], DT, isOutput=True)
        with TileContext(nc) as tc:
            with (
                tc.tile_pool(name="persist", bufs=1) as pp,
                tc.tile_pool(name="scrp", bufs=scr_bufs) as sp,
            ):
                self.scr_pool = sp
                m_tile = pp.tile([P, 16 * FD], DT, tag="m_stage", name="m_stage")
                out_tile = pp.tile([P, 8 * FD], DT, tag="out_stage", name="out_stage")
                v_tiles = [pp.tile([P, FD], DT, tag=f"v{j}", name=f"v{j}") for j in range(16)]
                bias_m10 = pp.tile([P, 1], DT, tag="bias_m10", name="bias_m10")
                bias_m5 = pp.tile([P, 1], DT, tag="bias_m5", name="bias_m5")
                nc.vector.memset(bias_m10[:], -10.0)
                nc.vector.memset(bias_m5[:], -5.0)
                self.bias_m10_ap = lambda: bias_m10[:]
                self.bias_m5_ap = lambda: bias_m5[:]
                self.v_aps = [(lambda jj=j: v_tiles[jj][:]) for j in range(16)]
                self.m_aps = [(lambda jj=j: m_tile[:][:, jj::16]) for j in range(16)]

                for blk in range(BLOCKS):
                    r0 = blk * BLOCK_ROWS
                    in_ap = self.msg[r0:r0 + BLOCK_ROWS, :].rearrange("(p f) w -> p (f w)", p=P)
                    nc.sync.dma_start(out=m_tile[:], in_=in_ap)
                    state = [Val(const=IV[j]) for j in range(8)]
                    for rnd in range(ROUNDS):
                        vals = {}
                        for j in range(8):
                            vals[j] = state[j]
                            vals[8 + j] = Val(const=IV[j])
                        for grp in (G_SCHEDULE[:4], G_SCHEDULE[4:]):
                            lanes = [[] for _ in grp]
                            for li, (a, b, c, d, xi, yi) in enumerate(grp):
                                self.begin_lane(lanes[li], li)
                                self.G(vals, a, b, c, d, xi, yi)
                                self.end_lane()
                            self.merge_lanes(lanes)
                        last = rnd == ROUNDS - 1
                        new_state = []
                        lanes = [[] for _ in range(8)]
                        for j in range(8):
                            self.begin_lane(lanes[j], j % 4)
                            if last:
                                dst = (lambda jj=j: out_tile[:][:, jj::8])
                                new_state.append(self.soft_xor(None, vals[j], vals[8 + j], dst_ap=dst))
                            else:
                                new_state.append(self.soft_xor(j, vals[j], vals[8 + j]))
                            self.end_lane()
                        self.merge_lanes(lanes)
                        state = new_state
                    out_ap = self.out[r0:r0 + BLOCK_ROWS, :].rearrange("(p f) w -> p (f w)", p=P)
                    nc.sync.dma_start(out=out_ap, in_=out_tile[:])
        hoist_excess_waits(nc)
        return nc


def hoist_excess_waits(nc, max_waits=1):
    """Walrus can't encode >~2 sync waits per instruction; move excess into
    standalone NoOps (1 wait each) right before the instruction."""
    n_hoisted = 0
    for f in nc.m.functions:
        for blk in f.blocks:
            need = False
            for inst in blk.instructions:
                si = inst.sync_info
                if si is not None and len(si.on_wait) > max_waits:
                    need = True
                    break
            if not need:
                continue
            newl = []
            for inst in blk.instructions:
                si = inst.sync_info
                if si is not None and len(si.on_wait) > max_waits:
                    conds = list(si.on_wait)
                    keep = conds[-max_waits:]
                    for c in conds[:-max_waits]:
                        nop = mybir.InstNoOp(
                            name=nc.get_next_instruction_name(), ins=[], outs=[])
                        nop.engine = inst.engine
                        _bass_rust.wait_op(
                            nop, SemaphoreHandle(c.ant_name, c.id),
                            c.wait_value, "sem-ge", False)
                        newl.append(nop)
                        n_hoisted += 1
                    inst.sync_info = mybir.SyncInfo(on_wait=keep, on_update=list(si.on_update))
                newl.append(inst)
            blk.instructions = newl
    return n_hoisted


def build_program():
    p = Program2()
    nc = p.build()
    return nc, p


# ----------------------------------------------------------------- entry
_cache = {}


def _get_nc():
    if "nc" not in _cache:
        _cache["nc"] = build_program()[0]
    return _cache["nc"]


def kernel(message, _trace=False):
    """Full (2000000, 16) f32 in -> (2000000, 8) f32 out, 8-core data parallel."""
    from concourse.bass_utils import run_bass_kernel_spmd
    msg = np.ascontiguousarray(np.asarray(message, dtype=np.float32))
    nc = _get_nc()
    pad = PAD_ROWS - msg.shape[0]
    msgp = np.concatenate([msg, np.zeros((pad, 16), np.float32)]) if pad > 0 else msg
    shards = msgp.reshape(N_CORES, CORE_ROWS, 16)
    in_maps = [{"message": shards[i]} for i in range(N_CORES)]
    kw = dict(trace=True) if _trace else {}
    res = run_bass_kernel_spmd(nc, in_maps, core_ids=list(range(N_CORES)), **kw)
    out = np.concatenate([res.results[i]["out"] for i in range(N_CORES)], axis=0)
    if _trace:
        _cache["last_result"] = res
    return out[: msg.shape[0]]



# revision 9
# speedup vs baseline: 1.8003x; 1.8003x over previous
"""Blake2 soft-cipher Bass kernel for Trainium2 (8 NeuronCores, data parallel).

v2: affine-deferred values + interval-tracked tiny-value elimination +
3-engine (DVE/ACT/Pool) load balancing.

Key numerical facts (all verified against f32 reference semantics):
- rot16/24/32 of any soft_xor output are exact scales by 2^-n (the wrapped
  fraction is identically zero because xor outputs are >= ~0.0132 > 2^-9).
- rot32-scaled values (<= 2^-32) are sub-half-ULP against every downstream
  addend (c-words >= 0.0131 => half-ulp >= 2^-31 > 2^-32), and shift sigmoid
  inputs by < 0.03 ulp: both uses collapse to "the tiny operand is invisible".
- sigmoid of near-const inputs (width of the sigma output interval < ~2e-6)
  is replaced by its midpoint constant; the xor against a constant xs
  factors into a quadratic in ys: 1 - (a1+b1*ys)(a2+b2*ys).
- rot63(x) = 2x - [x>=0.5] exactly, up to a dropped x*2^-63 term that only
  survives rounding when frac(2x) == 0 (measure-zero, magnitude 5e-20).
- soft_xor / soft_add results carry deferred affine (scale, bias): biases
  fold into ACT sigmoid bias constants and stt scalars for free.
"""
import sys
sys.path.insert(0, "/opt/trn_rl_repo")
import math
import numpy as np
from concourse import bass, mybir
from concourse.tile import TileContext
from concourse.bass_primitives_rust import SemaphoreHandle
from concourse.bass import _bass_rust

A = mybir.AluOpType
F = mybir.ActivationFunctionType
DT = mybir.dt.float32

# ---------------------------------------------------------------- geometry
P = 128
FD = 980
BLOCK_ROWS = P * FD
BLOCKS = 2
CORE_ROWS = BLOCK_ROWS * BLOCKS
N_CORES = 8
TOTAL_ROWS = 2_000_000
PAD_ROWS = CORE_ROWS * N_CORES

ROUNDS = 10
G_SCHEDULE = [
    (0, 4, 8, 12, 0, 1), (1, 5, 9, 13, 2, 3), (2, 6, 10, 14, 4, 5), (3, 7, 11, 15, 6, 7),
    (0, 5, 10, 15, 8, 9), (1, 6, 11, 12, 10, 11), (2, 7, 8, 13, 12, 13), (3, 4, 9, 14, 14, 15),
]
_IV_INTS = [7640891576956012808, 13503953896175478587, 4354685564936845355,
            11912009170470909681, 5840696475078001361, 11170449401992604703,
            2270897969802886507, 6620516959819538809]
IV = (np.asarray(_IV_INTS, dtype=np.float32) / np.float32(2.0**64)).astype(np.float32)

EPSB = 3e-6            # interval widening for hw sigmoid inexactness
DROP_ADD_TOL = 2.5e-7  # drop soft_add addends with |value| below this
XS_CONST_TOL = 3e-6    # sigma-output interval width below which xs is const

POOL_TT = True         # allow Pool engine for tensor_tensor add/sub/mult


def configure(fd=980, blocks=2):
    global FD, BLOCK_ROWS, BLOCKS, CORE_ROWS, PAD_ROWS
    FD = fd
    BLOCKS = blocks
    BLOCK_ROWS = P * FD
    CORE_ROWS = BLOCK_ROWS * BLOCKS
    PAD_ROWS = CORE_ROWS * N_CORES


def f32(x):
    return np.float32(x)


def sig64(z):
    z = float(z)
    if z >= 0:
        return 1.0 / (1.0 + math.exp(-z))
    e = math.exp(z)
    return e / (1.0 + e)


def sigc(z):
    """f32-rounded sigmoid of f64 arg."""
    return float(np.float32(sig64(z)))


# cost estimates (per-op ns at current FD), mirroring the TRN2 cost model
def ns_tt():
    return (FD + 58) / 0.96

def ns_ts():
    return (FD / 2 + 58) / 0.96

def ns_act():
    return (FD + 352) / 1.2

def ns_pool_tt():
    return (FD / 1.2) / 0.42 + 95


class Val:
    """true_value = scale * ap[...] + bias, or a build-time const.
    lo/hi bound the TRUE value."""
    __slots__ = ("const", "ap", "scale", "bias", "lo", "hi")

    def __init__(self, const=None, ap=None, scale=1.0, bias=0.0, lo=None, hi=None):
        self.const = const
        self.ap = ap
        self.scale = float(scale)
        self.bias = float(bias)
        if const is not None:
            self.lo = self.hi = float(const)
        else:
            assert lo is not None and hi is not None, "tensor Val needs bounds"
            self.lo = float(lo)
            self.hi = float(hi)

    @property
    def is_const(self):
        return self.const is not None

    def absmax(self):
        return max(abs(self.lo), abs(self.hi))


def sadd_const(a, b):
    """f32-faithful soft_add of two consts."""
    s = f32(f32(a) + f32(b))
    z = f32(f32(10.0) * f32(s - f32(1.0)))
    w = f32(sig64(float(z)))
    return float(f32(s - w))


def sxor_const(x, y):
    xs = f32(sig64(float(f32(f32(10.0) * f32(f32(x) - f32(0.5))))))
    ys = f32(sig64(float(f32(f32(10.0) * f32(f32(y) - f32(0.5))))))
    t1 = f32(xs * f32(f32(1.0) - ys))
    t2 = f32(f32(f32(1.0) - xs) * ys)
    r = f32(f32(t1 + t2) - f32(t1 * t2))
    return float(min(max(float(r), 0.0), 1.0))


def rot63_const(c):
    c = f32(c)
    m = f32(math.floor(float(c) * 2.0))  # floor(2c) for c in [0,1)
    sl = f32(f32(c * f32(2.0**65)) - f32(m * f32(2.0**64)))
    s = f32(f32(c * f32(2.0)) + sl)
    # mod(s, 2^64) with s < 2^64 is identity
    return float(f32(s / f32(2.0**64)))


def half_ulp_floor(x):
    """smallest half-ulp among f32 values >= x (x > 0)."""
    assert x > 0
    _, e = math.frexp(x)  # x = m * 2^e, m in [0.5, 1)
    return 2.0 ** (e - 25)


class Prog:
    def __init__(self):
        self.nc = bass.Bass("TRN2")
        self.est = {"dve": 0.0, "act": 0.0, "pool": 0.0}
        self._lane = None
        self._lane_id = 0
        self._bias_tiles = {}
        self._bias_pool = None
        self._bias_count = 0

    # ---------------- lane machinery (same as baseline)
    def _run(self, fn):
        if self._lane is not None:
            self._lane.append(fn)
        else:
            fn()

    def begin_lane(self, lane, lane_id=0):
        self._lane = lane
        self._lane_id = lane_id

    def end_lane(self):
        self._lane = None

    def merge_lanes(self, lanes):
        lanes = [list(l) for l in lanes if l]
        while lanes:
            nxt = []
            for l in lanes:
                l.pop(0)()
                if l:
                    nxt.append(l)
            lanes = nxt

    # ---------------- scratch
    def scr(self):
        cell = {}
        tag = f"scr{self._lane_id or 0}"
        def get():
            if "t" not in cell:
                cell["t"] = self.scr_pool.tile([P, FD], DT, tag=tag, name=tag, bufs=3)
            return cell["t"][:]
        return get

    # ---------------- bias const tiles for ACT sigmoid
    def bias_ap(self, value):
        v = float(np.float32(value))
        if v not in self._bias_tiles:
            t = self._bias_pool.tile([P, 1], DT, tag=f"bias{self._bias_count}",
                                     name=f"bias{self._bias_count}")
            self._bias_count += 1
            # eager emit (before any lane-deferred consumer is flushed)
            self.nc.vector.memset(t[:], v)
            self.est["dve"] += 65.0
            self._bias_tiles[v] = t
        t = self._bias_tiles[v]
        return lambda: t[:]

    # ---------------- balanced emitters
    def tt(self, out, a, b, op):
        """tensor_tensor; Pool-eligible for add/sub/mult."""
        pool_ok = POOL_TT and op in (A.add, A.subtract, A.mult)
        cd, cp = ns_tt(), ns_pool_tt()
        if pool_ok and self.est["pool"] + cp < self.est["dve"] + cd:
            self.est["pool"] += cp
            self._run(lambda: self.nc.gpsimd.tensor_tensor(out(), a(), b(), op=op))
        else:
            self.est["dve"] += cd
            self._run(lambda: self.nc.vector.tensor_tensor(out(), a(), b(), op=op))

    def stt(self, out, in0, scalar, in1, op0, op1, rev0=False):
        def fn():
            i = self.nc.vector.scalar_tensor_tensor(
                out(), in0(), float(scalar), in1(), op0=op0, op1=op1)
            if rev0:
                i.ins.reverse0 = True
        self.est["dve"] += ns_tt()
        self._run(fn)

    def ts_cmp(self, out, in0, thresh, op0):
        """compare tensor_scalar — DVE only."""
        self.est["dve"] += ns_ts()
        self._run(lambda: self.nc.vector.tensor_scalar(out(), in0(), float(thresh), None, op0=op0))

    def affine(self, out, in0, scale, bias, strided=False):
        """out = scale*in0 + bias on DVE-ts or ACT-copy, balancer's choice."""
        scale = float(np.float32(scale))
        bias = float(np.float32(bias))
        cd = ns_tt() if strided else ns_ts()
        ca = ns_act()
        if self.est["act"] + ca < self.est["dve"] + cd:
            self.est["act"] += ca
            self._run(lambda: self.nc.scalar.activation(out(), in0(), F.Copy,
                                                        bias=bias, scale=scale))
        else:
            self.est["dve"] += cd
            if bias == 0.0:
                self._run(lambda: self.nc.vector.tensor_scalar(out(), in0(), scale, None, op0=A.mult))
            else:
                self._run(lambda: self.nc.vector.tensor_scalar(out(), in0(), scale, bias,
                                                               op0=A.mult, op1=A.add))

    def affine1m(self, out, in0):
        """out = 1 - in0."""
        cd, ca = ns_ts(), ns_act()
        if self.est["act"] + ca < self.est["dve"] + cd:
            self.est["act"] += ca
            self._run(lambda: self.nc.scalar.activation(out(), in0(), F.Copy,
                                                        bias=1.0, scale=-1.0))
        else:
            self.est["dve"] += cd
            def fn():
                i = self.nc.vector.tensor_scalar(out(), in0(), 1.0, None, op0=A.subtract)
                i.ins.reverse0 = True
            self._run(fn)

    def sigmoid(self, out, in_ap, scale, biasval):
        """out = sigmoid(scale * in + biasval); biasval via const tile."""
        b = self.bias_ap(biasval)
        s = float(np.float32(scale))
        self.est["act"] += ns_act()
        self._run(lambda: self.nc.scalar.activation(out(), in_ap(), F.Sigmoid,
                                                    bias=b(), scale=s))

    # ---------------- interval helpers
    def sig_interval(self, V):
        """interval of sigmoid(10*(V-0.5)) over V's bounds (widened)."""
        if V.is_const:
            x = sigc(f32(f32(10.0) * f32(f32(V.const) - f32(0.5))))
            return (x, x)
        lo = sig64(10.0 * (V.lo - EPSB - 0.5)) - 1e-7
        hi = sig64(10.0 * (V.hi + EPSB - 0.5)) + 1e-7
        return (lo, hi)

    # ---------------- soft primitives
    def soft_add(self, dst_slot, Av, Bv, dst_ap=None):
        if Av.is_const and Bv.is_const:
            return Val(const=sadd_const(Av.const, Bv.const))

        # skip/drop tiny addend (exactness or tolerance based)
        for X, Y in ((Av, Bv), (Bv, Av)):
            if Y.is_const or Y.absmax() > 2e-5:
                continue
            ymax = Y.absmax()
            exact_ok = (X.lo > 1e-30) and (ymax < 0.99 * half_ulp_floor(X.lo))
            if ymax <= DROP_ADD_TOL or exact_ok:
                if X.is_const:
                    return Val(const=sadd_const(X.const, 0.0))
                return self._sadd_finish(dst_slot, X.ap, X.scale, X.bias,
                                         Av, Bv, dst_ap)

        if Av.is_const or Bv.is_const:
            c, T = (Av.const, Bv) if Av.is_const else (Bv.const, Av)
            return self._sadd_finish(dst_slot, T.ap, T.scale, T.bias + c,
                                     Av, Bv, dst_ap)

        # combine two tensors
        sa, sb = Av.scale, Bv.scale
        h = self.scr()
        if sa == sb:
            self.tt(h, Av.ap, Bv.ap, A.add)
            ss = sa
        elif sa == -sb:
            self.tt(h, Av.ap, Bv.ap, A.subtract)
            ss = sa
        else:
            # keep |ratio| <= 1: fold the smaller-scale operand in scaled form
            if abs(sa) <= abs(sb):
                self.stt(h, Av.ap, sa / sb, Bv.ap, A.mult, A.add)
                ss = sb
            else:
                self.stt(h, Bv.ap, sb / sa, Av.ap, A.mult, A.add)
                ss = sa
        return self._sadd_finish(dst_slot, h, ss, Av.bias + Bv.bias, Av, Bv, dst_ap)

    def _sadd_finish(self, dst_slot, h, ss, beta, Av, Bv, dst_ap):
        # bounds of true output
        slo, shi = Av.lo + Bv.lo, Av.hi + Bv.hi
        lo = slo - sig64(10.0 * (shi - 1.0)) - EPSB
        hi = shi - sig64(10.0 * (slo - 1.0)) + EPSB
        w = self.scr()
        self.sigmoid(w, h, 10.0 * ss, 10.0 * beta - 10.0)
        dst = dst_ap if dst_ap is not None else self.v_aps[dst_slot]
        if ss == 1.0:
            self.tt(dst, h, w, A.subtract)
        else:
            self.stt(dst, h, ss, w, A.mult, A.subtract)
        return Val(ap=dst, scale=1.0, bias=beta, lo=lo, hi=hi)

    def soft_xor(self, dst_slot, Xv, Yv, dst_ap=None, materialize=False):
        if Xv.is_const and Yv.is_const:
            return Val(const=sxor_const(Xv.const, Yv.const))

        xi = self.sig_interval(Xv)
        yi = self.sig_interval(Yv)
        x_constish = Xv.is_const or (xi[1] - xi[0] <= XS_CONST_TOL)
        y_constish = Yv.is_const or (yi[1] - yi[0] <= XS_CONST_TOL)

        if x_constish and y_constish:
            # both sides' sigmoids constant: output is a build-time const
            xbar, ybar = (xi[0] + xi[1]) / 2, (yi[0] + yi[1]) / 2
            t1 = xbar * (1.0 - ybar)
            t2 = (1.0 - xbar) * ybar
            return Val(const=float(f32(t1 + t2 - t1 * t2)))
        if x_constish:
            return self._sxor_const_side(dst_slot, xi, Yv, yi, dst_ap, materialize)
        if y_constish:
            return self._sxor_const_side(dst_slot, yi, Xv, xi, dst_ap, materialize)
        return self._sxor_full(dst_slot, Xv, xi, Yv, yi, dst_ap, materialize)

    def _xor_bounds(self, xi, yi):
        corners = [(a, b) for a in xi for b in yi]
        vals = [a + b - 2 * a * b for a, b in corners]
        m, M = min(vals), max(vals)
        lo = max(0.0, m - m * m / 4.0) - EPSB
        hi = min(M, 1.0) + EPSB
        return lo, hi

    def _sxor_const_side(self, dst_slot, xi, Yv, yi, dst_ap, materialize):
        """xs is effectively const xbar; out = 1 - (a1+b1*ys)(a2+b2*ys)."""
        xbar = (xi[0] + xi[1]) / 2
        a1, b1 = 1.0 - xbar, xbar
        a2, b2 = 1.0, -(1.0 - xbar)
        c0 = a1 * a2
        c1 = a1 * b2 + b1 * a2
        c2 = b1 * b2
        lo, hi = self._xor_bounds(xi, yi)
        ys = self.scr()
        self.sigmoid(ys, Yv.ap, 10.0 * Yv.scale, 10.0 * Yv.bias - 5.0)
        t = self.scr()
        self.affine(t, ys, c2, c1)
        dst = dst_ap if dst_ap is not None else self.v_aps[dst_slot]
        if materialize:
            q = self.scr()
            self.tt(q, t, ys, A.mult)
            self.affine(dst, q, -1.0, 1.0 - c0, strided=True)
            return Val(ap=dst, scale=1.0, bias=0.0, lo=lo, hi=hi)
        self.tt(dst, t, ys, A.mult)
        return Val(ap=dst, scale=-1.0, bias=1.0 - c0, lo=lo, hi=hi)

    def _sxor_full(self, dst_slot, Xv, xi, Yv, yi, dst_ap, materialize):
        lo, hi = self._xor_bounds(xi, yi)
        xs = self.scr()
        ys = self.scr()
        t1 = self.scr()
        self.sigmoid(xs, Xv.ap, 10.0 * Xv.scale, 10.0 * Xv.bias - 5.0)
        self.sigmoid(ys, Yv.ap, 10.0 * Yv.scale, 10.0 * Yv.bias - 5.0)
        self.stt(t1, ys, 1.0, xs, A.subtract, A.mult, rev0=True)   # (1-ys)*xs
        self.stt(xs, xs, 1.0, ys, A.subtract, A.mult, rev0=True)   # xs <- t2=(1-xs)*ys
        self.affine1m(ys, xs)                                      # ys <- 1-t2
        dst = dst_ap if dst_ap is not None else self.v_aps[dst_slot]
        if materialize:
            self.stt(t1, t1, 1.0, ys, A.subtract, A.mult, rev0=True)  # (1-t1)(1-t2)
            self.affine(dst, t1, -1.0, 1.0, strided=True)
            return Val(ap=dst, scale=1.0, bias=0.0, lo=lo, hi=hi)
        self.stt(dst, t1, 1.0, ys, A.subtract, A.mult, rev0=True)
        return Val(ap=dst, scale=-1.0, bias=1.0, lo=lo, hi=hi)

    def rotate(self, slot, n, V):
        if V.is_const:
            if n in (16, 24, 32):
                return Val(const=float(f32(V.const)) * 2.0 ** (-n))
            assert n == 63
            return Val(const=rot63_const(V.const))
        if n in (16, 24, 32):
            need = {16: 2.0**-25, 24: 2.0**-17, 32: 2.0**-9}[n]
            assert V.lo >= need, f"rot{n} scale-defer needs lo>={need}, got {V.lo}"
            k = 2.0 ** (-n)
            return Val(ap=V.ap, scale=V.scale * k, bias=V.bias * k,
                       lo=V.lo * k, hi=V.hi * k)
        assert n == 63
        m = self.scr()
        dst = self.v_aps[slot]
        s, b = V.scale, V.bias
        # mask = [X >= 0.5] with X = s*u + b
        if s < 0:
            self.ts_cmp(m, V.ap, (b - 0.5) / (-s), A.is_le)
        else:
            self.ts_cmp(m, V.ap, (0.5 - b) / s, A.is_ge)
        # r = 2X - m = (2s)*u - m, bias 2b deferred
        self.stt(dst, V.ap, 2.0 * s, m, A.mult, A.subtract)
        return Val(ap=dst, scale=1.0, bias=2.0 * b, lo=-EPSB, hi=1.0 + EPSB)

    # ---------------- G function
    def G(self, vals, a, b, c, d, xi, yi):
        mx = Val(ap=self.m_aps[xi], lo=0.0, hi=1.0)
        my = Val(ap=self.m_aps[yi], lo=0.0, hi=1.0)
        vals[a] = self.soft_add(a, vals[a], vals[b])
        vals[a] = self.soft_add(a, vals[a], mx)
        vals[d] = self.soft_xor(d, vals[d], vals[a])
        vals[d] = self.rotate(d, 32, vals[d])
        vals[c] = self.soft_add(c, vals[c], vals[d])
        vals[b] = self.soft_xor(b, vals[b], vals[c])
        vals[b] = self.rotate(b, 24, vals[b])
        vals[a] = self.soft_add(a, vals[a], vals[b])
        vals[a] = self.soft_add(a, vals[a], my)
        vals[d] = self.soft_xor(d, vals[d], vals[a])
        vals[d] = self.rotate(d, 16, vals[d])
        vals[c] = self.soft_add(c, vals[c], vals[d])
        vals[b] = self.soft_xor(b, vals[b], vals[c])
        vals[b] = self.rotate(b, 63, vals[b])

    # ---------------- whole program
    def build(self, scr_bufs=10):
        nc = self.nc
        self.msg = nc.declare_dram_parameter("message", [CORE_ROWS, 16], DT, isOutput=False)
        self.out = nc.declare_dram_parameter("out", [CORE_ROWS, 8], DT, isOutput=True)
        with TileContext(nc) as tc:
            with (
                tc.tile_pool(name="persist", bufs=1) as pp,
                tc.tile_pool(name="scrp", bufs=scr_bufs) as sp,
            ):
                self.scr_pool = sp
                self._bias_pool = pp
                m_tile = pp.tile([P, 16 * FD], DT, tag="m_stage", name="m_stage")
                out_tile = pp.tile([P, 8 * FD], DT, tag="out_stage", name="out_stage")
                v_tiles = [pp.tile([P, FD], DT, tag=f"v{j}", name=f"v{j}") for j in range(16)]
                self.v_aps = [(lambda jj=j: v_tiles[jj][:]) for j in range(16)]
                self.m_aps = [(lambda jj=j: m_tile[:][:, jj::16]) for j in range(16)]

                for blk in range(BLOCKS):
                    r0 = blk * BLOCK_ROWS
                    in_ap = self.msg[r0:r0 + BLOCK_ROWS, :].rearrange("(p f) w -> p (f w)", p=P)
                    nc.sync.dma_start(out=m_tile[:], in_=in_ap)
                    state = [Val(const=float(IV[j])) for j in range(8)]
                    for rnd in range(ROUNDS):
                        vals = {}
                        for j in range(8):
                            vals[j] = state[j]
                            vals[8 + j] = Val(const=float(IV[j]))
                        for grp in (G_SCHEDULE[:4], G_SCHEDULE[4:]):
                            lanes = [[] for _ in grp]
                            for li, (a, b, c, d, gx, gy) in enumerate(grp):
                                self.begin_lane(lanes[li], li)
                                self.G(vals, a, b, c, d, gx, gy)
                                self.end_lane()
                            self.merge_lanes(lanes)
                        last = rnd == ROUNDS - 1
                        new_state = [None] * 8
                        # concatenate j and j+4 into one lane per scratch tag so
                        # every tag sees a single sequential alloc stream
                        lanes = [[] for _ in range(4)]
                        for li in range(4):
                            self.begin_lane(lanes[li], li)
                            for j in (li, li + 4):
                                if last:
                                    dst = (lambda jj=j: out_tile[:][:, jj::8])
                                    new_state[j] = self.soft_xor(
                                        None, vals[j], vals[8 + j], dst_ap=dst,
                                        materialize=True)
                                else:
                                    new_state[j] = self.soft_xor(j, vals[j], vals[8 + j])
                            self.end_lane()
                        self.merge_lanes(lanes)
                        state = new_state
                    out_ap = self.out[r0:r0 + BLOCK_ROWS, :].rearrange("(p f) w -> p (f w)", p=P)
                    nc.sync.dma_start(out=out_ap, in_=out_tile[:])
        hoist_excess_waits(nc)
        return nc


def hoist_excess_waits(nc, max_waits=1):
    """Walrus can't encode >~2 sync waits per instruction; move excess into
    standalone NoOps (1 wait each) right before the instruction."""
    n_hoisted = 0
    for fu in nc.m.functions:
        for blk in fu.blocks:
            need = False
            for inst in blk.instructions:
                si = inst.sync_info
                if si is not None and len(si.on_wait) > max_waits:
                    need = True
                    break
            if not need:
                continue
            newl = []
            for inst in blk.instructions:
                si = inst.sync_info
                if si is not None and len(si.on_wait) > max_waits:
                    conds = list(si.on_wait)
                    keep = conds[-max_waits:]
                    for cnd in conds[:-max_waits]:
                        nop = mybir.InstNoOp(
                            name=nc.get_next_instruction_name(), ins=[], outs=[])
                        nop.engine = inst.engine
                        _bass_rust.wait_op(
                            nop, SemaphoreHandle(cnd.ant_name, cnd.id),
                            cnd.wait_value, "sem-ge", False)
                        newl.append(nop)
                        n_hoisted += 1
                    inst.sync_info = mybir.SyncInfo(on_wait=keep, on_update=list(si.on_update))
                newl.append(inst)
            blk.instructions = newl
    return n_hoisted


def build_program():
    p = Prog()
    nc = p.build()
    return nc, p


# ----------------------------------------------------------------- entry
_cache = {}


def _get_nc():
    if "nc" not in _cache:
        _cache["nc"] = build_program()[0]
    return _cache["nc"]


def kernel(message, _trace=False):
    """Full (2000000, 16) f32 in -> (2000000, 8) f32 out, 8-core data parallel."""
    from concourse.bass_utils import run_bass_kernel_spmd
    msg = np.ascontiguousarray(np.asarray(message, dtype=np.float32))
    nc = _get_nc()
    pad = PAD_ROWS - msg.shape[0]
    msgp = np.concatenate([msg, np.zeros((pad, 16), np.float32)]) if pad > 0 else msg
    shards = msgp.reshape(N_CORES, CORE_ROWS, 16)
    in_maps = [{"message": shards[i]} for i in range(N_CORES)]
    kw = dict(trace=True) if _trace else {}
    res = run_bass_kernel_spmd(nc, in_maps, core_ids=list(range(N_CORES)), **kw)
    out = np.concatenate([res.results[i]["out"] for i in range(N_CORES)], axis=0)
    if _trace:
        _cache["last_result"] = res
    return out[: msg.shape[0]]


# revision 11
# speedup vs baseline: 2.2778x; 1.2653x over previous
"""Blake2 soft-cipher Bass kernel for Trainium2 (8 NeuronCores, data parallel).

v2: affine-deferred values + interval-tracked tiny-value elimination +
3-engine (DVE/ACT/Pool) load balancing.

Key numerical facts (all verified against f32 reference semantics):
- rot16/24/32 of any soft_xor output are exact scales by 2^-n (the wrapped
  fraction is identically zero because xor outputs are >= ~0.0132 > 2^-9).
- rot32-scaled values (<= 2^-32) are sub-half-ULP against every downstream
  addend (c-words >= 0.0131 => half-ulp >= 2^-31 > 2^-32), and shift sigmoid
  inputs by < 0.03 ulp: both uses collapse to "the tiny operand is invisible".
- sigmoid of near-const inputs (width of the sigma output interval < ~2e-6)
  is replaced by its midpoint constant; the xor against a constant xs
  factors into a quadratic in ys: 1 - (a1+b1*ys)(a2+b2*ys).
- rot63(x) = 2x - [x>=0.5] exactly, up to a dropped x*2^-63 term that only
  survives rounding when frac(2x) == 0 (measure-zero, magnitude 5e-20).
- soft_xor / soft_add results carry deferred affine (scale, bias): biases
  fold into ACT sigmoid bias constants and stt scalars for free.
"""
import sys
sys.path.insert(0, "/opt/trn_rl_repo")
import math
import numpy as np
from concourse import bass, mybir
from concourse.tile import TileContext
from concourse.bass_primitives_rust import SemaphoreHandle
from concourse.bass import _bass_rust

A = mybir.AluOpType
F = mybir.ActivationFunctionType
DT = mybir.dt.float32

# ---------------------------------------------------------------- geometry
P = 128
FD = 980
BLOCK_ROWS = P * FD
BLOCKS = 2
CORE_ROWS = BLOCK_ROWS * BLOCKS
N_CORES = 8
TOTAL_ROWS = 2_000_000
PAD_ROWS = CORE_ROWS * N_CORES

ROUNDS = 10
G_SCHEDULE = [
    (0, 4, 8, 12, 0, 1), (1, 5, 9, 13, 2, 3), (2, 6, 10, 14, 4, 5), (3, 7, 11, 15, 6, 7),
    (0, 5, 10, 15, 8, 9), (1, 6, 11, 12, 10, 11), (2, 7, 8, 13, 12, 13), (3, 4, 9, 14, 14, 15),
]
_IV_INTS = [7640891576956012808, 13503953896175478587, 4354685564936845355,
            11912009170470909681, 5840696475078001361, 11170449401992604703,
            2270897969802886507, 6620516959819538809]
IV = (np.asarray(_IV_INTS, dtype=np.float32) / np.float32(2.0**64)).astype(np.float32)

EPSB = 3e-6            # interval widening for hw sigmoid inexactness
DROP_ADD_TOL = 2.5e-7  # drop soft_add addends with |value| below this
XS_CONST_TOL = 3e-6    # sigma-output interval width below which xs is const

POOL_TT = True         # allow Pool engine for tensor_tensor add/sub/mult


def configure(fd=980, blocks=2):
    global FD, BLOCK_ROWS, BLOCKS, CORE_ROWS, PAD_ROWS
    FD = fd
    BLOCKS = blocks
    BLOCK_ROWS = P * FD
    CORE_ROWS = BLOCK_ROWS * BLOCKS
    PAD_ROWS = CORE_ROWS * N_CORES


def f32(x):
    return np.float32(x)


def sig64(z):
    z = float(z)
    if z >= 0:
        return 1.0 / (1.0 + math.exp(-z))
    e = math.exp(z)
    return e / (1.0 + e)


def sigc(z):
    """f32-rounded sigmoid of f64 arg."""
    return float(np.float32(sig64(z)))


# cost estimates (per-op ns at current FD), mirroring the TRN2 cost model
def ns_tt():
    return (FD + 58) / 0.96

def ns_ts():
    return (FD / 2 + 58) / 0.96

def ns_act():
    return (FD + 352) / 1.2

def ns_pool_tt():
    return (FD / 1.2) / 0.42 + 95


class Val:
    """true_value = scale * ap[...] + bias, or a build-time const.
    lo/hi bound the TRUE value."""
    __slots__ = ("const", "ap", "scale", "bias", "lo", "hi")

    def __init__(self, const=None, ap=None, scale=1.0, bias=0.0, lo=None, hi=None):
        self.const = const
        self.ap = ap
        self.scale = float(scale)
        self.bias = float(bias)
        if const is not None:
            self.lo = self.hi = float(const)
        else:
            assert lo is not None and hi is not None, "tensor Val needs bounds"
            self.lo = float(lo)
            self.hi = float(hi)

    @property
    def is_const(self):
        return self.const is not None

    def absmax(self):
        return max(abs(self.lo), abs(self.hi))


def sadd_const(a, b):
    """f32-faithful soft_add of two consts."""
    s = f32(f32(a) + f32(b))
    z = f32(f32(10.0) * f32(s - f32(1.0)))
    w = f32(sig64(float(z)))
    return float(f32(s - w))


def sxor_const(x, y):
    xs = f32(sig64(float(f32(f32(10.0) * f32(f32(x) - f32(0.5))))))
    ys = f32(sig64(float(f32(f32(10.0) * f32(f32(y) - f32(0.5))))))
    t1 = f32(xs * f32(f32(1.0) - ys))
    t2 = f32(f32(f32(1.0) - xs) * ys)
    r = f32(f32(t1 + t2) - f32(t1 * t2))
    return float(min(max(float(r), 0.0), 1.0))


def rot63_const(c):
    c = f32(c)
    m = f32(math.floor(float(c) * 2.0))  # floor(2c) for c in [0,1)
    sl = f32(f32(c * f32(2.0**65)) - f32(m * f32(2.0**64)))
    s = f32(f32(c * f32(2.0)) + sl)
    # mod(s, 2^64) with s < 2^64 is identity
    return float(f32(s / f32(2.0**64)))


def half_ulp_floor(x):
    """smallest half-ulp among f32 values >= x (x > 0)."""
    assert x > 0
    _, e = math.frexp(x)  # x = m * 2^e, m in [0.5, 1)
    return 2.0 ** (e - 25)


class Prog:
    def __init__(self):
        self.nc = bass.Bass("TRN2")
        self.est = {"dve": 0.0, "act": 0.0, "pool": 0.0}
        self._lane = None
        self._lane_id = 0
        self._bias_tiles = {}
        self._bias_pool = None
        self._bias_count = 0

    # ---------------- lane machinery (same as baseline)
    def _run(self, fn):
        if self._lane is not None:
            self._lane.append(fn)
        else:
            fn()

    def begin_lane(self, lane, lane_id=0):
        self._lane = lane
        self._lane_id = lane_id

    def end_lane(self):
        self._lane = None

    def merge_lanes(self, lanes):
        lanes = [list(l) for l in lanes if l]
        while lanes:
            nxt = []
            for l in lanes:
                l.pop(0)()
                if l:
                    nxt.append(l)
            lanes = nxt

    # ---------------- scratch
    def scr(self):
        cell = {}
        tag = f"scr{self._lane_id or 0}"
        def get():
            if "t" not in cell:
                cell["t"] = self.scr_pool.tile([P, FD], DT, tag=tag, name=tag, bufs=3)
            return cell["t"][:]
        return get

    # ---------------- bias const tiles for ACT sigmoid
    def bias_ap(self, value):
        v = float(np.float32(value))
        if v not in self._bias_tiles:
            t = self._bias_pool.tile([P, 1], DT, tag=f"bias{self._bias_count}",
                                     name=f"bias{self._bias_count}")
            self._bias_count += 1
            # eager emit (before any lane-deferred consumer is flushed)
            self.nc.vector.memset(t[:], v)
            self.est["dve"] += 65.0
            self._bias_tiles[v] = t
        t = self._bias_tiles[v]
        return lambda: t[:]

    # ---------------- balanced emitters
    def tt(self, out, a, b, op):
        """tensor_tensor; Pool-eligible for add/sub/mult."""
        pool_ok = POOL_TT and op in (A.add, A.subtract, A.mult)
        cd, cp = ns_tt(), ns_pool_tt()
        if pool_ok and self.est["pool"] + cp < self.est["dve"] + cd:
            self.est["pool"] += cp
            self._run(lambda: self.nc.gpsimd.tensor_tensor(out(), a(), b(), op=op))
        else:
            self.est["dve"] += cd
            self._run(lambda: self.nc.vector.tensor_tensor(out(), a(), b(), op=op))

    def stt(self, out, in0, scalar, in1, op0, op1, rev0=False):
        def fn():
            i = self.nc.vector.scalar_tensor_tensor(
                out(), in0(), float(scalar), in1(), op0=op0, op1=op1)
            if rev0:
                i.ins.reverse0 = True
        self.est["dve"] += ns_tt()
        self._run(fn)

    def ts_cmp(self, out, in0, thresh, op0):
        """compare tensor_scalar — DVE only."""
        self.est["dve"] += ns_ts()
        self._run(lambda: self.nc.vector.tensor_scalar(out(), in0(), float(thresh), None, op0=op0))

    def affine(self, out, in0, scale, bias, strided=False):
        """out = scale*in0 + bias on DVE-ts or ACT-copy, balancer's choice."""
        scale = float(np.float32(scale))
        bias = float(np.float32(bias))
        cd = ns_tt() if strided else ns_ts()
        ca = ns_act()
        if self.est["act"] + ca < self.est["dve"] + cd:
            self.est["act"] += ca
            self._run(lambda: self.nc.scalar.activation(out(), in0(), F.Copy,
                                                        bias=bias, scale=scale))
        else:
            self.est["dve"] += cd
            if bias == 0.0:
                self._run(lambda: self.nc.vector.tensor_scalar(out(), in0(), scale, None, op0=A.mult))
            else:
                self._run(lambda: self.nc.vector.tensor_scalar(out(), in0(), scale, bias,
                                                               op0=A.mult, op1=A.add))

    def affine1m(self, out, in0):
        """out = 1 - in0."""
        cd, ca = ns_ts(), ns_act()
        if self.est["act"] + ca < self.est["dve"] + cd:
            self.est["act"] += ca
            self._run(lambda: self.nc.scalar.activation(out(), in0(), F.Copy,
                                                        bias=1.0, scale=-1.0))
        else:
            self.est["dve"] += cd
            def fn():
                i = self.nc.vector.tensor_scalar(out(), in0(), 1.0, None, op0=A.subtract)
                i.ins.reverse0 = True
            self._run(fn)

    def sigmoid(self, out, in_ap, scale, biasval):
        """out = sigmoid(scale * in + biasval); biasval via const tile."""
        b = self.bias_ap(biasval)
        s = float(np.float32(scale))
        self.est["act"] += ns_act()
        self._run(lambda: self.nc.scalar.activation(out(), in_ap(), F.Sigmoid,
                                                    bias=b(), scale=s))

    # ---------------- interval helpers
    def sig_interval(self, V):
        """interval of sigmoid(10*(V-0.5)) over V's bounds (widened)."""
        if V.is_const:
            x = sigc(f32(f32(10.0) * f32(f32(V.const) - f32(0.5))))
            return (x, x)
        lo = sig64(10.0 * (V.lo - EPSB - 0.5)) - 1e-7
        hi = sig64(10.0 * (V.hi + EPSB - 0.5)) + 1e-7
        return (lo, hi)

    # ---------------- soft primitives
    def soft_add(self, dst_slot, Av, Bv, dst_ap=None):
        if Av.is_const and Bv.is_const:
            return Val(const=sadd_const(Av.const, Bv.const))

        # skip/drop tiny addend (exactness or tolerance based)
        for X, Y in ((Av, Bv), (Bv, Av)):
            if Y.is_const or Y.absmax() > 2e-5:
                continue
            ymax = Y.absmax()
            exact_ok = (X.lo > 1e-30) and (ymax < 0.99 * half_ulp_floor(X.lo))
            if ymax <= DROP_ADD_TOL or exact_ok:
                if X.is_const:
                    return Val(const=sadd_const(X.const, 0.0))
                return self._sadd_finish(dst_slot, X.ap, X.scale, X.bias,
                                         Av, Bv, dst_ap)

        if Av.is_const or Bv.is_const:
            c, T = (Av.const, Bv) if Av.is_const else (Bv.const, Av)
            return self._sadd_finish(dst_slot, T.ap, T.scale, T.bias + c,
                                     Av, Bv, dst_ap)

        # combine two tensors
        sa, sb = Av.scale, Bv.scale
        h = self.scr()
        if sa == sb:
            self.tt(h, Av.ap, Bv.ap, A.add)
            ss = sa
        elif sa == -sb:
            self.tt(h, Av.ap, Bv.ap, A.subtract)
            ss = sa
        else:
            # keep |ratio| <= 1: fold the smaller-scale operand in scaled form
            if abs(sa) <= abs(sb):
                self.stt(h, Av.ap, sa / sb, Bv.ap, A.mult, A.add)
                ss = sb
            else:
                self.stt(h, Bv.ap, sb / sa, Av.ap, A.mult, A.add)
                ss = sa
        return self._sadd_finish(dst_slot, h, ss, Av.bias + Bv.bias, Av, Bv, dst_ap)

    def _sadd_finish(self, dst_slot, h, ss, beta, Av, Bv, dst_ap):
        # bounds of true output
        slo, shi = Av.lo + Bv.lo, Av.hi + Bv.hi
        lo = slo - sig64(10.0 * (shi - 1.0)) - EPSB
        hi = shi - sig64(10.0 * (slo - 1.0)) + EPSB
        w = self.scr()
        self.sigmoid(w, h, 10.0 * ss, 10.0 * beta - 10.0)
        dst = dst_ap if dst_ap is not None else self.v_aps[dst_slot]
        if ss == 1.0:
            self.tt(dst, h, w, A.subtract)
        else:
            self.stt(dst, h, ss, w, A.mult, A.subtract)
        return Val(ap=dst, scale=1.0, bias=beta, lo=lo, hi=hi)

    def soft_xor(self, dst_slot, Xv, Yv, dst_ap=None, materialize=False):
        if Xv.is_const and Yv.is_const:
            return Val(const=sxor_const(Xv.const, Yv.const))

        xi = self.sig_interval(Xv)
        yi = self.sig_interval(Yv)
        x_constish = Xv.is_const or (xi[1] - xi[0] <= XS_CONST_TOL)
        y_constish = Yv.is_const or (yi[1] - yi[0] <= XS_CONST_TOL)

        if x_constish and y_constish:
            # both sides' sigmoids constant: output is a build-time const
            xbar, ybar = (xi[0] + xi[1]) / 2, (yi[0] + yi[1]) / 2
            t1 = xbar * (1.0 - ybar)
            t2 = (1.0 - xbar) * ybar
            return Val(const=float(f32(t1 + t2 - t1 * t2)))
        if x_constish:
            return self._sxor_const_side(dst_slot, xi, Yv, yi, dst_ap, materialize)
        if y_constish:
            return self._sxor_const_side(dst_slot, yi, Xv, xi, dst_ap, materialize)
        return self._sxor_full(dst_slot, Xv, xi, Yv, yi, dst_ap, materialize)

    def _xor_bounds(self, xi, yi):
        corners = [(a, b) for a in xi for b in yi]
        vals = [a + b - 2 * a * b for a, b in corners]
        m, M = min(vals), max(vals)
        lo = max(0.0, m - m * m / 4.0) - EPSB
        hi = min(M, 1.0) + EPSB
        return lo, hi

    def _sxor_const_side(self, dst_slot, xi, Yv, yi, dst_ap, materialize):
        """xs is effectively const xbar; out = 1 - (a1+b1*ys)(a2+b2*ys)."""
        xbar = (xi[0] + xi[1]) / 2
        a1, b1 = 1.0 - xbar, xbar
        a2, b2 = 1.0, -(1.0 - xbar)
        c0 = a1 * a2
        c1 = a1 * b2 + b1 * a2
        c2 = b1 * b2
        lo, hi = self._xor_bounds(xi, yi)
        ys = self.scr()
        self.sigmoid(ys, Yv.ap, 10.0 * Yv.scale, 10.0 * Yv.bias - 5.0)
        t = self.scr()
        self.affine(t, ys, c2, c1)
        dst = dst_ap if dst_ap is not None else self.v_aps[dst_slot]
        if materialize:
            q = self.scr()
            self.tt(q, t, ys, A.mult)
            self.affine(dst, q, -1.0, 1.0 - c0, strided=True)
            return Val(ap=dst, scale=1.0, bias=0.0, lo=lo, hi=hi)
        self.tt(dst, t, ys, A.mult)
        return Val(ap=dst, scale=-1.0, bias=1.0 - c0, lo=lo, hi=hi)

    def _sxor_full(self, dst_slot, Xv, xi, Yv, yi, dst_ap, materialize):
        lo, hi = self._xor_bounds(xi, yi)
        xs = self.scr()
        ys = self.scr()
        t1 = self.scr()
        self.sigmoid(xs, Xv.ap, 10.0 * Xv.scale, 10.0 * Xv.bias - 5.0)
        self.sigmoid(ys, Yv.ap, 10.0 * Yv.scale, 10.0 * Yv.bias - 5.0)
        self.stt(t1, ys, 1.0, xs, A.subtract, A.mult, rev0=True)   # (1-ys)*xs
        self.stt(xs, xs, 1.0, ys, A.subtract, A.mult, rev0=True)   # xs <- t2=(1-xs)*ys
        self.affine1m(ys, xs)                                      # ys <- 1-t2
        dst = dst_ap if dst_ap is not None else self.v_aps[dst_slot]
        if materialize:
            self.stt(t1, t1, 1.0, ys, A.subtract, A.mult, rev0=True)  # (1-t1)(1-t2)
            self.affine(dst, t1, -1.0, 1.0, strided=True)
            return Val(ap=dst, scale=1.0, bias=0.0, lo=lo, hi=hi)
        self.stt(dst, t1, 1.0, ys, A.subtract, A.mult, rev0=True)
        return Val(ap=dst, scale=-1.0, bias=1.0, lo=lo, hi=hi)

    def soft_xor_dead(self, Xv, Yv):
        """soft_xor whose RUNTIME value is never consumed (only its bounds
        feed later const-folds). Emits nothing; returns a phantom Val whose
        ap raises if ever dereferenced."""
        if Xv.is_const and Yv.is_const:
            return Val(const=sxor_const(Xv.const, Yv.const))
        xi = self.sig_interval(Xv)
        yi = self.sig_interval(Yv)
        lo, hi = self._xor_bounds(xi, yi)
        def phantom():
            raise AssertionError("phantom (value-dead) soft_xor output was dereferenced")
        return Val(ap=phantom, scale=-1.0, bias=1.0, lo=lo, hi=hi)

    def rotate(self, slot, n, V):
        if V.is_const:
            if n in (16, 24, 32):
                return Val(const=float(f32(V.const)) * 2.0 ** (-n))
            assert n == 63
            return Val(const=rot63_const(V.const))
        if n in (16, 24, 32):
            need = {16: 2.0**-25, 24: 2.0**-17, 32: 2.0**-9}[n]
            assert V.lo >= need, f"rot{n} scale-defer needs lo>={need}, got {V.lo}"
            k = 2.0 ** (-n)
            return Val(ap=V.ap, scale=V.scale * k, bias=V.bias * k,
                       lo=V.lo * k, hi=V.hi * k)
        assert n == 63
        m = self.scr()
        dst = self.v_aps[slot]
        s, b = V.scale, V.bias
        # mask = [X >= 0.5] with X = s*u + b
        if s < 0:
            self.ts_cmp(m, V.ap, (b - 0.5) / (-s), A.is_le)
        else:
            self.ts_cmp(m, V.ap, (0.5 - b) / s, A.is_ge)
        # r = 2X - m = (2s)*u - m, bias 2b deferred
        self.stt(dst, V.ap, 2.0 * s, m, A.mult, A.subtract)
        return Val(ap=dst, scale=1.0, bias=2.0 * b, lo=-EPSB, hi=1.0 + EPSB)

    # ---------------- G function
    def G(self, vals, a, b, c, d, xi, yi):
        mx = Val(ap=self.m_aps[xi], lo=0.0, hi=1.0)
        my = Val(ap=self.m_aps[yi], lo=0.0, hi=1.0)
        vals[a] = self.soft_add(a, vals[a], vals[b])
        vals[a] = self.soft_add(a, vals[a], mx)
        # #3's output only survives rot32 (sub-half-ULP everywhere) -> its
        # runtime value is dead; bounds still feed #5's skip and #10's consts
        vals[d] = self.soft_xor_dead(vals[d], vals[a])
        vals[d] = self.rotate(d, 32, vals[d])
        vals[c] = self.soft_add(c, vals[c], vals[d])
        # #6's output only survives rot24: dropped by #8 (<=2.5e-7) and
        # const-folded by #13 -> value-dead as well
        vals[b] = self.soft_xor_dead(vals[b], vals[c])
        vals[b] = self.rotate(b, 24, vals[b])
        vals[a] = self.soft_add(a, vals[a], vals[b])
        vals[a] = self.soft_add(a, vals[a], my)
        vals[d] = self.soft_xor(d, vals[d], vals[a])
        vals[d] = self.rotate(d, 16, vals[d])
        vals[c] = self.soft_add(c, vals[c], vals[d])
        vals[b] = self.soft_xor(b, vals[b], vals[c])
        vals[b] = self.rotate(b, 63, vals[b])

    # ---------------- whole program
    def build(self, scr_bufs=10):
        nc = self.nc
        self.msg = nc.declare_dram_parameter("message", [CORE_ROWS, 16], DT, isOutput=False)
        self.out = nc.declare_dram_parameter("out", [CORE_ROWS, 8], DT, isOutput=True)
        with TileContext(nc) as tc:
            with (
                tc.tile_pool(name="persist", bufs=1) as pp,
                tc.tile_pool(name="scrp", bufs=scr_bufs) as sp,
            ):
                self.scr_pool = sp
                self._bias_pool = pp
                m_tile = pp.tile([P, 16 * FD], DT, tag="m_stage", name="m_stage")
                out_tile = pp.tile([P, 8 * FD], DT, tag="out_stage", name="out_stage")
                v_tiles = [pp.tile([P, FD], DT, tag=f"v{j}", name=f"v{j}") for j in range(16)]
                self.v_aps = [(lambda jj=j: v_tiles[jj][:]) for j in range(16)]
                self.m_aps = [(lambda jj=j: m_tile[:][:, jj::16]) for j in range(16)]

                for blk in range(BLOCKS):
                    r0 = blk * BLOCK_ROWS
                    in_ap = self.msg[r0:r0 + BLOCK_ROWS, :].rearrange("(p f) w -> p (f w)", p=P)
                    nc.sync.dma_start(out=m_tile[:], in_=in_ap)
                    state = [Val(const=float(IV[j])) for j in range(8)]
                    for rnd in range(ROUNDS):
                        vals = {}
                        for j in range(8):
                            vals[j] = state[j]
                            vals[8 + j] = Val(const=float(IV[j]))
                        for grp in (G_SCHEDULE[:4], G_SCHEDULE[4:]):
                            lanes = [[] for _ in grp]
                            for li, (a, b, c, d, gx, gy) in enumerate(grp):
                                self.begin_lane(lanes[li], li)
                                self.G(vals, a, b, c, d, gx, gy)
                                self.end_lane()
                            self.merge_lanes(lanes)
                        last = rnd == ROUNDS - 1
                        new_state = [None] * 8
                        # concatenate j and j+4 into one lane per scratch tag so
                        # every tag sees a single sequential alloc stream
                        lanes = [[] for _ in range(4)]
                        for li in range(4):
                            self.begin_lane(lanes[li], li)
                            for j in (li, li + 4):
                                if last:
                                    dst = (lambda jj=j: out_tile[:][:, jj::8])
                                    new_state[j] = self.soft_xor(
                                        None, vals[j], vals[8 + j], dst_ap=dst,
                                        materialize=True)
                                else:
                                    new_state[j] = self.soft_xor(j, vals[j], vals[8 + j])
                            self.end_lane()
                        self.merge_lanes(lanes)
                        state = new_state
                    out_ap = self.out[r0:r0 + BLOCK_ROWS, :].rearrange("(p f) w -> p (f w)", p=P)
                    nc.sync.dma_start(out=out_ap, in_=out_tile[:])
        hoist_excess_waits(nc)
        return nc


def hoist_excess_waits(nc, max_waits=1):
    """Walrus can't encode >~2 sync waits per instruction; move excess into
    standalone NoOps (1 wait each) right before the instruction."""
    n_hoisted = 0
    for fu in nc.m.functions:
        for blk in fu.blocks:
            need = False
            for inst in blk.instructions:
                si = inst.sync_info
                if si is not None and len(si.on_wait) > max_waits:
                    need = True
                    break
            if not need:
                continue
            newl = []
            for inst in blk.instructions:
                si = inst.sync_info
                if si is not None and len(si.on_wait) > max_waits:
                    conds = list(si.on_wait)
                    keep = conds[-max_waits:]
                    for cnd in conds[:-max_waits]:
                        nop = mybir.InstNoOp(
                            name=nc.get_next_instruction_name(), ins=[], outs=[])
                        nop.engine = inst.engine
                        _bass_rust.wait_op(
                            nop, SemaphoreHandle(cnd.ant_name, cnd.id),
                            cnd.wait_value, "sem-ge", False)
                        newl.append(nop)
                        n_hoisted += 1
                    inst.sync_info = mybir.SyncInfo(on_wait=keep, on_update=list(si.on_update))
                newl.append(inst)
            blk.instructions = newl
    return n_hoisted


def build_program():
    p = Prog()
    nc = p.build()
    return nc, p


# ----------------------------------------------------------------- entry
_cache = {}


def _get_nc():
    if "nc" not in _cache:
        _cache["nc"] = build_program()[0]
    return _cache["nc"]


def kernel(message, _trace=False):
    """Full (2000000, 16) f32 in -> (2000000, 8) f32 out, 8-core data parallel."""
    from concourse.bass_utils import run_bass_kernel_spmd
    msg = np.ascontiguousarray(np.asarray(message, dtype=np.float32))
    nc = _get_nc()
    pad = PAD_ROWS - msg.shape[0]
    msgp = np.concatenate([msg, np.zeros((pad, 16), np.float32)]) if pad > 0 else msg
    shards = msgp.reshape(N_CORES, CORE_ROWS, 16)
    in_maps = [{"message": shards[i]} for i in range(N_CORES)]
    kw = dict(trace=True) if _trace else {}
    res = run_bass_kernel_spmd(nc, in_maps, core_ids=list(range(N_CORES)), **kw)
    out = np.concatenate([res.results[i]["out"] for i in range(N_CORES)], axis=0)
    if _trace:
        _cache["last_result"] = res
    return out[: msg.shape[0]]


# revision 18
# speedup vs baseline: 5.2839x; 2.3197x over previous
"""Blake2 soft-cipher Bass kernel for Trainium2 (8 NeuronCores, data parallel).

v2: affine-deferred values + interval-tracked tiny-value elimination +
3-engine (DVE/ACT/Pool) load balancing.

Key numerical facts (all verified against f32 reference semantics):
- rot16/24/32 of any soft_xor output are exact scales by 2^-n (the wrapped
  fraction is identically zero because xor outputs are >= ~0.0132 > 2^-9).
- rot32-scaled values (<= 2^-32) are sub-half-ULP against every downstream
  addend (c-words >= 0.0131 => half-ulp >= 2^-31 > 2^-32), and shift sigmoid
  inputs by < 0.03 ulp: both uses collapse to "the tiny operand is invisible".
- sigmoid of near-const inputs (width of the sigma output interval < ~2e-6)
  is replaced by its midpoint constant; the xor against a constant xs
  factors into a quadratic in ys: 1 - (a1+b1*ys)(a2+b2*ys).
- rot63(x) = 2x - [x>=0.5] exactly, up to a dropped x*2^-63 term that only
  survives rounding when frac(2x) == 0 (measure-zero, magnitude 5e-20).
- soft_xor / soft_add results carry deferred affine (scale, bias): biases
  fold into ACT sigmoid bias constants and stt scalars for free.
"""
import sys
sys.path.insert(0, "/opt/trn_rl_repo")
import math
import numpy as np
from concourse import bass, mybir
from concourse.tile import TileContext
from concourse.bass_primitives_rust import SemaphoreHandle
from concourse.bass import _bass_rust

A = mybir.AluOpType
F = mybir.ActivationFunctionType
DT = mybir.dt.float32

# ---------------------------------------------------------------- geometry
P = 128
FD = 980
BLOCK_ROWS = P * FD
BLOCKS = 2
CORE_ROWS = BLOCK_ROWS * BLOCKS
N_CORES = 8
TOTAL_ROWS = 2_000_000
PAD_ROWS = CORE_ROWS * N_CORES

ROUNDS = 10
G_SCHEDULE = [
    (0, 4, 8, 12, 0, 1), (1, 5, 9, 13, 2, 3), (2, 6, 10, 14, 4, 5), (3, 7, 11, 15, 6, 7),
    (0, 5, 10, 15, 8, 9), (1, 6, 11, 12, 10, 11), (2, 7, 8, 13, 12, 13), (3, 4, 9, 14, 14, 15),
]
_IV_INTS = [7640891576956012808, 13503953896175478587, 4354685564936845355,
            11912009170470909681, 5840696475078001361, 11170449401992604703,
            2270897969802886507, 6620516959819538809]
IV = (np.asarray(_IV_INTS, dtype=np.float32) / np.float32(2.0**64)).astype(np.float32)

EPSB = 3e-6            # interval widening for hw sigmoid inexactness
DROP_ADD_TOL = 2.5e-7  # drop soft_add addends with |value| below this
XS_CONST_TOL = 3e-6    # sigma-output interval width below which xs is const

POOL_TT = True         # allow Pool engine for tensor_tensor add/sub/mult
DROP_D16 = True        # drop the 1.5e-5 rot16 addend into c (kills the whole
                       # runtime d-lineage; error budget ~1e-4 vs 2e-2 gate)
DROP_D16_TOL = 2e-5


def configure(fd=980, blocks=2):
    global FD, BLOCK_ROWS, BLOCKS, CORE_ROWS, PAD_ROWS
    FD = fd
    BLOCKS = blocks
    BLOCK_ROWS = P * FD
    CORE_ROWS = BLOCK_ROWS * BLOCKS
    PAD_ROWS = CORE_ROWS * N_CORES


def f32(x):
    return np.float32(x)


def sig64(z):
    z = float(z)
    if z >= 0:
        return 1.0 / (1.0 + math.exp(-z))
    e = math.exp(z)
    return e / (1.0 + e)


def sigc(z):
    """f32-rounded sigmoid of f64 arg."""
    return float(np.float32(sig64(z)))


# cost estimates (per-op ns at current FD), mirroring the TRN2 cost model
def ns_tt():
    return (FD + 58) / 0.96

def ns_ts():
    return (FD / 2 + 58) / 0.96

def ns_act():
    return (FD + 352) / 1.2

def ns_pool_tt():
    return (FD / 1.2) / 0.42 + 95


class Val:
    """true_value = scale * ap[...] + bias, or a build-time const.
    lo/hi bound the TRUE value."""
    __slots__ = ("const", "ap", "scale", "bias", "lo", "hi")

    def __init__(self, const=None, ap=None, scale=1.0, bias=0.0, lo=None, hi=None):
        self.const = const
        self.ap = ap
        self.scale = float(scale)
        self.bias = float(bias)
        if const is not None:
            self.lo = self.hi = float(const)
        else:
            assert lo is not None and hi is not None, "tensor Val needs bounds"
            self.lo = float(lo)
            self.hi = float(hi)

    @property
    def is_const(self):
        return self.const is not None

    def absmax(self):
        return max(abs(self.lo), abs(self.hi))


def sadd_const(a, b):
    """f32-faithful soft_add of two consts."""
    s = f32(f32(a) + f32(b))
    z = f32(f32(10.0) * f32(s - f32(1.0)))
    w = f32(sig64(float(z)))
    return float(f32(s - w))


def sxor_const(x, y):
    xs = f32(sig64(float(f32(f32(10.0) * f32(f32(x) - f32(0.5))))))
    ys = f32(sig64(float(f32(f32(10.0) * f32(f32(y) - f32(0.5))))))
    t1 = f32(xs * f32(f32(1.0) - ys))
    t2 = f32(f32(f32(1.0) - xs) * ys)
    r = f32(f32(t1 + t2) - f32(t1 * t2))
    return float(min(max(float(r), 0.0), 1.0))


def rot63_const(c):
    c = f32(c)
    m = f32(math.floor(float(c) * 2.0))  # floor(2c) for c in [0,1)
    sl = f32(f32(c * f32(2.0**65)) - f32(m * f32(2.0**64)))
    s = f32(f32(c * f32(2.0)) + sl)
    # mod(s, 2^64) with s < 2^64 is identity
    return float(f32(s / f32(2.0**64)))


def half_ulp_floor(x):
    """smallest half-ulp among f32 values >= x (x > 0)."""
    assert x > 0
    _, e = math.frexp(x)  # x = m * 2^e, m in [0.5, 1)
    return 2.0 ** (e - 25)


class Prog:
    def __init__(self):
        self.nc = bass.Bass("TRN2")
        self.est = {"dve": 0.0, "act": 0.0, "pool": 0.0}
        self._lane = None
        self._lane_id = 0
        self._bias_tiles = {}
        self._bias_pool = None
        self._bias_count = 0

    # ---------------- lane machinery (same as baseline)
    def _run(self, fn):
        if self._lane is not None:
            self._lane.append(fn)
        else:
            fn()

    def begin_lane(self, lane, lane_id=0):
        self._lane = lane
        self._lane_id = lane_id

    def end_lane(self):
        self._lane = None

    def merge_lanes(self, lanes):
        lanes = [list(l) for l in lanes if l]
        while lanes:
            nxt = []
            for l in lanes:
                l.pop(0)()
                if l:
                    nxt.append(l)
            lanes = nxt

    # ---------------- scratch
    def scr(self):
        cell = {}
        tag = f"scr{self._lane_id or 0}"
        def get():
            if "t" not in cell:
                cell["t"] = self.scr_pool.tile([P, FD], DT, tag=tag, name=tag, bufs=3)
            return cell["t"][:]
        return get

    # ---------------- bias const tiles for ACT sigmoid
    def bias_ap(self, value):
        v = float(np.float32(value))
        if v not in self._bias_tiles:
            t = self._bias_pool.tile([P, 1], DT, tag=f"bias{self._bias_count}",
                                     name=f"bias{self._bias_count}")
            self._bias_count += 1
            # eager emit (before any lane-deferred consumer is flushed)
            self.nc.vector.memset(t[:], v)
            self.est["dve"] += 65.0
            self._bias_tiles[v] = t
        t = self._bias_tiles[v]
        return lambda: t[:]

    # ---------------- balanced emitters
    def tt(self, out, a, b, op):
        """tensor_tensor; Pool-eligible for add/sub/mult."""
        pool_ok = POOL_TT and op in (A.add, A.subtract, A.mult)
        cd, cp = ns_tt(), ns_pool_tt()
        if pool_ok and self.est["pool"] + cp < self.est["dve"] + cd:
            self.est["pool"] += cp
            self._run(lambda: self.nc.gpsimd.tensor_tensor(out(), a(), b(), op=op))
        else:
            self.est["dve"] += cd
            self._run(lambda: self.nc.vector.tensor_tensor(out(), a(), b(), op=op))

    def stt(self, out, in0, scalar, in1, op0, op1, rev0=False):
        def fn():
            i = self.nc.vector.scalar_tensor_tensor(
                out(), in0(), float(scalar), in1(), op0=op0, op1=op1)
            if rev0:
                i.ins.reverse0 = True
        self.est["dve"] += ns_tt()
        self._run(fn)

    def ts_cmp(self, out, in0, thresh, op0):
        """compare tensor_scalar — DVE only."""
        self.est["dve"] += ns_ts()
        self._run(lambda: self.nc.vector.tensor_scalar(out(), in0(), float(thresh), None, op0=op0))

    def affine(self, out, in0, scale, bias, strided=False):
        """out = scale*in0 + bias on DVE-ts or ACT-copy, balancer's choice."""
        scale = float(np.float32(scale))
        bias = float(np.float32(bias))
        cd = ns_tt() if strided else ns_ts()
        ca = ns_act()
        if self.est["act"] + ca < self.est["dve"] + cd:
            self.est["act"] += ca
            self._run(lambda: self.nc.scalar.activation(out(), in0(), F.Copy,
                                                        bias=bias, scale=scale))
        else:
            self.est["dve"] += cd
            if bias == 0.0:
                self._run(lambda: self.nc.vector.tensor_scalar(out(), in0(), scale, None, op0=A.mult))
            else:
                self._run(lambda: self.nc.vector.tensor_scalar(out(), in0(), scale, bias,
                                                               op0=A.mult, op1=A.add))

    def affine1m(self, out, in0):
        """out = 1 - in0."""
        cd, ca = ns_ts(), ns_act()
        if self.est["act"] + ca < self.est["dve"] + cd:
            self.est["act"] += ca
            self._run(lambda: self.nc.scalar.activation(out(), in0(), F.Copy,
                                                        bias=1.0, scale=-1.0))
        else:
            self.est["dve"] += cd
            def fn():
                i = self.nc.vector.tensor_scalar(out(), in0(), 1.0, None, op0=A.subtract)
                i.ins.reverse0 = True
            self._run(fn)

    def sigmoid(self, out, in_ap, scale, biasval):
        """out = sigmoid(scale * in + biasval); biasval via const tile."""
        b = self.bias_ap(biasval)
        s = float(np.float32(scale))
        self.est["act"] += ns_act()
        self._run(lambda: self.nc.scalar.activation(out(), in_ap(), F.Sigmoid,
                                                    bias=b(), scale=s))

    # ---------------- interval helpers
    def sig_interval(self, V):
        """interval of sigmoid(10*(V-0.5)) over V's bounds (widened)."""
        if V.is_const:
            x = sigc(f32(f32(10.0) * f32(f32(V.const) - f32(0.5))))
            return (x, x)
        lo = sig64(10.0 * (V.lo - EPSB - 0.5)) - 1e-7
        hi = sig64(10.0 * (V.hi + EPSB - 0.5)) + 1e-7
        return (lo, hi)

    # ---------------- soft primitives
    def soft_add(self, dst_slot, Av, Bv, dst_ap=None):
        if Av.is_const and Bv.is_const:
            return Val(const=sadd_const(Av.const, Bv.const))

        # skip/drop tiny addend (exactness or tolerance based)
        for X, Y in ((Av, Bv), (Bv, Av)):
            if Y.is_const or Y.absmax() > 2e-5:
                continue
            ymax = Y.absmax()
            exact_ok = (X.lo > 1e-30) and (ymax < 0.99 * half_ulp_floor(X.lo))
            drop_tol = DROP_D16_TOL if DROP_D16 else DROP_ADD_TOL
            if ymax <= drop_tol or exact_ok:
                if X.is_const:
                    return Val(const=sadd_const(X.const, 0.0))
                return self._sadd_finish(dst_slot, X.ap, X.scale, X.bias,
                                         Av, Bv, dst_ap)

        if Av.is_const or Bv.is_const:
            c, T = (Av.const, Bv) if Av.is_const else (Bv.const, Av)
            return self._sadd_finish(dst_slot, T.ap, T.scale, T.bias + c,
                                     Av, Bv, dst_ap)

        # combine two tensors
        sa, sb = Av.scale, Bv.scale
        h = self.scr()
        if sa == sb:
            self.tt(h, Av.ap, Bv.ap, A.add)
            ss = sa
        elif sa == -sb:
            self.tt(h, Av.ap, Bv.ap, A.subtract)
            ss = sa
        else:
            # keep |ratio| <= 1: fold the smaller-scale operand in scaled form
            if abs(sa) <= abs(sb):
                self.stt(h, Av.ap, sa / sb, Bv.ap, A.mult, A.add)
                ss = sb
            else:
                self.stt(h, Bv.ap, sb / sa, Av.ap, A.mult, A.add)
                ss = sa
        return self._sadd_finish(dst_slot, h, ss, Av.bias + Bv.bias, Av, Bv, dst_ap)

    def _sadd_finish(self, dst_slot, h, ss, beta, Av, Bv, dst_ap):
        # bounds of true output
        slo, shi = Av.lo + Bv.lo, Av.hi + Bv.hi
        lo = slo - sig64(10.0 * (shi - 1.0)) - EPSB
        hi = shi - sig64(10.0 * (slo - 1.0)) + EPSB
        w = self.scr()
        self.sigmoid(w, h, 10.0 * ss, 10.0 * beta - 10.0)
        dst = dst_ap if dst_ap is not None else self.v_aps[dst_slot]
        if ss == 1.0:
            self.tt(dst, h, w, A.subtract)
        else:
            self.stt(dst, h, ss, w, A.mult, A.subtract)
        return Val(ap=dst, scale=1.0, bias=beta, lo=lo, hi=hi)

    def soft_xor(self, dst_slot, Xv, Yv, dst_ap=None, materialize=False):
        if Xv.is_const and Yv.is_const:
            return Val(const=sxor_const(Xv.const, Yv.const))

        xi = self.sig_interval(Xv)
        yi = self.sig_interval(Yv)
        x_constish = Xv.is_const or (xi[1] - xi[0] <= XS_CONST_TOL)
        y_constish = Yv.is_const or (yi[1] - yi[0] <= XS_CONST_TOL)

        if x_constish and y_constish:
            # both sides' sigmoids constant: output is a build-time const
            xbar, ybar = (xi[0] + xi[1]) / 2, (yi[0] + yi[1]) / 2
            t1 = xbar * (1.0 - ybar)
            t2 = (1.0 - xbar) * ybar
            return Val(const=float(f32(t1 + t2 - t1 * t2)))
        if x_constish:
            return self._sxor_const_side(dst_slot, xi, Yv, yi, dst_ap, materialize)
        if y_constish:
            return self._sxor_const_side(dst_slot, yi, Xv, xi, dst_ap, materialize)
        return self._sxor_full(dst_slot, Xv, xi, Yv, yi, dst_ap, materialize)

    def _xor_bounds(self, xi, yi):
        corners = [(a, b) for a in xi for b in yi]
        vals = [a + b - 2 * a * b for a, b in corners]
        m, M = min(vals), max(vals)
        lo = max(0.0, m - m * m / 4.0) - EPSB
        hi = min(M, 1.0) + EPSB
        return lo, hi

    def _sxor_const_side(self, dst_slot, xi, Yv, yi, dst_ap, materialize):
        """xs is effectively const xbar; out = 1 - (a1+b1*ys)(a2+b2*ys)."""
        # canonicalize tiny-class midpoints so emitted constants (and ACT
        # bias tiles) repeat across rounds instead of tracking each
        # instance's slightly-different interval
        if xi[1] - xi[0] > 0:
            if xi[1] <= sig64(10 * (3e-10 - 0.5)) + 1e-6:
                xbar = sigc(10.0 * (2.0**-33 - 0.5))
            elif xi[1] <= sig64(10 * (3e-7 - 0.5)) + 1e-6:
                xbar = sigc(10.0 * (2.0**-25 - 0.5))
            elif xi[1] <= sig64(10 * (3e-5 - 0.5)) + 1e-6:
                xbar = sigc(10.0 * (2.0**-17 - 0.5))
            else:
                xbar = (xi[0] + xi[1]) / 2
        else:
            xbar = (xi[0] + xi[1]) / 2
        a1, b1 = 1.0 - xbar, xbar
        a2, b2 = 1.0, -(1.0 - xbar)
        c0 = a1 * a2
        c1 = a1 * b2 + b1 * a2
        c2 = b1 * b2
        lo, hi = self._xor_bounds(xi, yi)
        ys = self.scr()
        self.sigmoid(ys, Yv.ap, 10.0 * Yv.scale, 10.0 * Yv.bias - 5.0)
        dst = dst_ap if dst_ap is not None else self.v_aps[dst_slot]
        # Square-ACT variant: q = c2*ys^2 + c1*ys = b^2 - (s*ys + bq)^2 with
        # s = sqrt(-c2), bq = -c1/(2s); out = (1-c0-bq^2) + S. Pure-ACT (2 ops)
        # vs sigma + affine + tt. Pick by projected engine load.
        sq_s = math.sqrt(-c2)
        sq_b = -c1 / (2.0 * sq_s)
        ca, cd_extra = ns_act(), ns_ts() + ns_tt()
        use_sq = (not materialize) and (
            self.est["act"] + ca <= min(self.est["dve"] + cd_extra,
                                        self.est["pool"] + ns_pool_tt() + ns_ts()))
        if use_sq:
            b_ap = self.bias_ap(sq_b)
            self.est["act"] += ca
            self._run(lambda: self.nc.scalar.activation(
                dst(), ys(), F.Square, bias=b_ap(), scale=float(np.float32(sq_s))))
            return Val(ap=dst, scale=1.0, bias=1.0 - c0 - sq_b * sq_b, lo=lo, hi=hi)
        t = self.scr()
        self.affine(t, ys, c2, c1)
        if materialize:
            q = self.scr()
            self.tt(q, t, ys, A.mult)
            self.affine(dst, q, -1.0, 1.0 - c0, strided=True)
            return Val(ap=dst, scale=1.0, bias=0.0, lo=lo, hi=hi)
        self.tt(dst, t, ys, A.mult)
        return Val(ap=dst, scale=-1.0, bias=1.0 - c0, lo=lo, hi=hi)

    def _sxor_full(self, dst_slot, Xv, xi, Yv, yi, dst_ap, materialize):
        lo, hi = self._xor_bounds(xi, yi)
        xs = self.scr()
        ys = self.scr()
        t1 = self.scr()
        self.sigmoid(xs, Xv.ap, 10.0 * Xv.scale, 10.0 * Xv.bias - 5.0)
        self.sigmoid(ys, Yv.ap, 10.0 * Yv.scale, 10.0 * Yv.bias - 5.0)
        self.stt(t1, ys, 1.0, xs, A.subtract, A.mult, rev0=True)   # (1-ys)*xs
        self.stt(xs, xs, 1.0, ys, A.subtract, A.mult, rev0=True)   # xs <- t2=(1-xs)*ys
        self.affine1m(ys, xs)                                      # ys <- 1-t2
        dst = dst_ap if dst_ap is not None else self.v_aps[dst_slot]
        if materialize:
            self.stt(t1, t1, 1.0, ys, A.subtract, A.mult, rev0=True)  # (1-t1)(1-t2)
            self.affine(dst, t1, -1.0, 1.0, strided=True)
            return Val(ap=dst, scale=1.0, bias=0.0, lo=lo, hi=hi)
        self.stt(dst, t1, 1.0, ys, A.subtract, A.mult, rev0=True)
        return Val(ap=dst, scale=-1.0, bias=1.0, lo=lo, hi=hi)

    def soft_xor_dead(self, Xv, Yv):
        """soft_xor whose RUNTIME value is never consumed (only its bounds
        feed later const-folds). Emits nothing; returns a phantom Val whose
        ap raises if ever dereferenced."""
        if Xv.is_const and Yv.is_const:
            return Val(const=sxor_const(Xv.const, Yv.const))
        xi = self.sig_interval(Xv)
        yi = self.sig_interval(Yv)
        lo, hi = self._xor_bounds(xi, yi)
        def phantom():
            raise AssertionError("phantom (value-dead) soft_xor output was dereferenced")
        return Val(ap=phantom, scale=-1.0, bias=1.0, lo=lo, hi=hi)

    def rotate(self, slot, n, V):
        if V.is_const:
            if n in (16, 24, 32):
                return Val(const=float(f32(V.const)) * 2.0 ** (-n))
            assert n == 63
            return Val(const=rot63_const(V.const))
        if n in (16, 24, 32):
            need = {16: 2.0**-25, 24: 2.0**-17, 32: 2.0**-9}[n]
            assert V.lo >= need, f"rot{n} scale-defer needs lo>={need}, got {V.lo}"
            k = 2.0 ** (-n)
            return Val(ap=V.ap, scale=V.scale * k, bias=V.bias * k,
                       lo=V.lo * k, hi=V.hi * k)
        assert n == 63
        m = self.scr()
        dst = self.v_aps[slot]
        s, b = V.scale, V.bias
        # mask = [X >= 0.5] with X = s*u + b
        if s < 0:
            self.ts_cmp(m, V.ap, (b - 0.5) / (-s), A.is_le)
        else:
            self.ts_cmp(m, V.ap, (0.5 - b) / s, A.is_ge)
        # r = 2X - m = (2s)*u - m, bias 2b deferred
        self.stt(dst, V.ap, 2.0 * s, m, A.mult, A.subtract)
        return Val(ap=dst, scale=1.0, bias=2.0 * b, lo=-EPSB, hi=1.0 + EPSB)

    # ---------------- G function
    def G(self, vals, a, b, c, d, xi, yi):
        mx = Val(ap=self.m_aps[xi], lo=0.0, hi=1.0)
        my = Val(ap=self.m_aps[yi], lo=0.0, hi=1.0)
        vals[a] = self.soft_add(a, vals[a], vals[b])
        vals[a] = self.soft_add(a, vals[a], mx)
        # #3's output only survives rot32 (sub-half-ULP everywhere) -> its
        # runtime value is dead; bounds still feed #5's skip and #10's consts
        vals[d] = self.soft_xor_dead(vals[d], vals[a])
        vals[d] = self.rotate(d, 32, vals[d])
        vals[c] = self.soft_add(c, vals[c], vals[d])
        # #6's output only survives rot24: dropped by #8 (<=2.5e-7) and
        # const-folded by #13 -> value-dead as well
        vals[b] = self.soft_xor_dead(vals[b], vals[c])
        vals[b] = self.rotate(b, 24, vals[b])
        vals[a] = self.soft_add(a, vals[a], vals[b])
        vals[a] = self.soft_add(a, vals[a], my)
        if DROP_D16:
            # with the d16 addend dropped in #12, #10's runtime value is
            # dead as well (its other consumers const-fold it)
            vals[d] = self.soft_xor_dead(vals[d], vals[a])
        else:
            vals[d] = self.soft_xor(d, vals[d], vals[a])
        vals[d] = self.rotate(d, 16, vals[d])
        vals[c] = self.soft_add(c, vals[c], vals[d])
        vals[b] = self.soft_xor(b, vals[b], vals[c])
        vals[b] = self.rotate(b, 63, vals[b])

    # ---------------- whole program
    def build(self, scr_bufs=10):
        nc = self.nc
        self.msg = nc.declare_dram_parameter("message", [CORE_ROWS, 16], DT, isOutput=False)
        self.out = nc.declare_dram_parameter("out", [CORE_ROWS, 8], DT, isOutput=True)
        with TileContext(nc) as tc:
            with (
                tc.tile_pool(name="persist", bufs=1) as pp,
                tc.tile_pool(name="scrp", bufs=scr_bufs) as sp,
            ):
                self.scr_pool = sp
                self._bias_pool = pp
                m_tile = pp.tile([P, 16 * FD], DT, tag="m_stage", name="m_stage")
                out_tile = pp.tile([P, 8 * FD], DT, tag="out_stage", name="out_stage")
                v_tiles = [pp.tile([P, FD], DT, tag=f"v{j}", name=f"v{j}") for j in range(16)]
                self.v_aps = [(lambda jj=j: v_tiles[jj][:]) for j in range(16)]
                self.m_aps = [(lambda jj=j: m_tile[:][:, jj::16]) for j in range(16)]

                for blk in range(BLOCKS):
                    r0 = blk * BLOCK_ROWS
                    in_ap = self.msg[r0:r0 + BLOCK_ROWS, :].rearrange("(p f) w -> p (f w)", p=P)
                    nc.sync.dma_start(out=m_tile[:], in_=in_ap)
                    state = [Val(const=float(IV[j])) for j in range(8)]
                    for rnd in range(ROUNDS):
                        vals = {}
                        for j in range(8):
                            vals[j] = state[j]
                            vals[8 + j] = Val(const=float(IV[j]))
                        for grp in (G_SCHEDULE[:4], G_SCHEDULE[4:]):
                            lanes = [[] for _ in grp]
                            for li, (a, b, c, d, gx, gy) in enumerate(grp):
                                self.begin_lane(lanes[li], li)
                                self.G(vals, a, b, c, d, gx, gy)
                                self.end_lane()
                            self.merge_lanes(lanes)
                        last = rnd == ROUNDS - 1
                        new_state = [None] * 8
                        # concatenate j and j+4 into one lane per scratch tag so
                        # every tag sees a single sequential alloc stream
                        lanes = [[] for _ in range(4)]
                        for li in range(4):
                            self.begin_lane(lanes[li], li)
                            for j in (li, li + 4):
                                if last:
                                    dst = (lambda jj=j: out_tile[:][:, jj::8])
                                    new_state[j] = self.soft_xor(
                                        None, vals[j], vals[8 + j], dst_ap=dst,
                                        materialize=True)
                                    if new_state[j].is_const:
                                        cv = float(np.float32(new_state[j].const))
                                        self._run(lambda dd=dst, vv=cv:
                                                  self.nc.vector.memset(dd(), vv))
                                        self.est["dve"] += ns_tt()
                                else:
                                    new_state[j] = self.soft_xor(j, vals[j], vals[8 + j])
                            self.end_lane()
                        self.merge_lanes(lanes)
                        state = new_state
                    out_ap = self.out[r0:r0 + BLOCK_ROWS, :].rearrange("(p f) w -> p (f w)", p=P)
                    nc.sync.dma_start(out=out_ap, in_=out_tile[:])
        hoist_excess_waits(nc)
        return nc


def hoist_excess_waits(nc, max_waits=1):
    """Walrus can't encode >~2 sync waits per instruction; move excess into
    standalone NoOps (1 wait each) right before the instruction."""
    n_hoisted = 0
    for fu in nc.m.functions:
        for blk in fu.blocks:
            need = False
            for inst in blk.instructions:
                si = inst.sync_info
                if si is not None and len(si.on_wait) > max_waits:
                    need = True
                    break
            if not need:
                continue
            newl = []
            for inst in blk.instructions:
                si = inst.sync_info
                if si is not None and len(si.on_wait) > max_waits:
                    conds = list(si.on_wait)
                    keep = conds[-max_waits:]
                    for cnd in conds[:-max_waits]:
                        nop = mybir.InstNoOp(
                            name=nc.get_next_instruction_name(), ins=[], outs=[])
                        nop.engine = inst.engine
                        _bass_rust.wait_op(
                            nop, SemaphoreHandle(cnd.ant_name, cnd.id),
                            cnd.wait_value, "sem-ge", False)
                        newl.append(nop)
                        n_hoisted += 1
                    inst.sync_info = mybir.SyncInfo(on_wait=keep, on_update=list(si.on_update))
                newl.append(inst)
            blk.instructions = newl
    return n_hoisted


def build_program():
    p = Prog()
    nc = p.build()
    return nc, p


# ----------------------------------------------------------------- entry
_cache = {}


def _get_nc():
    if "nc" not in _cache:
        _cache["nc"] = build_program()[0]
    return _cache["nc"]


def kernel(message, _trace=False):
    """Full (2000000, 16) f32 in -> (2000000, 8) f32 out, 8-core data parallel."""
    from concourse.bass_utils import run_bass_kernel_spmd
    msg = np.ascontiguousarray(np.asarray(message, dtype=np.float32))
    nc = _get_nc()
    pad = PAD_ROWS - msg.shape[0]
    msgp = np.concatenate([msg, np.zeros((pad, 16), np.float32)]) if pad > 0 else msg
    shards = msgp.reshape(N_CORES, CORE_ROWS, 16)
    in_maps = [{"message": shards[i]} for i in range(N_CORES)]
    kw = dict(trace=True) if _trace else {}
    res = run_bass_kernel_spmd(nc, in_maps, core_ids=list(range(N_CORES)), **kw)
    out = np.concatenate([res.results[i]["out"] for i in range(N_CORES)], axis=0)
    if _trace:
        _cache["last_result"] = res
    return out[: msg.shape[0]]


# revision 31
# speedup vs baseline: 5.4587x; 1.0331x over previous
"""Blake2 soft-cipher Bass kernel for Trainium2 (8 NeuronCores, data parallel).

v2: affine-deferred values + interval-tracked tiny-value elimination +
3-engine (DVE/ACT/Pool) load balancing.

Key numerical facts (all verified against f32 reference semantics):
- rot16/24/32 of any soft_xor output are exact scales by 2^-n (the wrapped
  fraction is identically zero because xor outputs are >= ~0.0132 > 2^-9).
- rot32-scaled values (<= 2^-32) are sub-half-ULP against every downstream
  addend (c-words >= 0.0131 => half-ulp >= 2^-31 > 2^-32), and shift sigmoid
  inputs by < 0.03 ulp: both uses collapse to "the tiny operand is invisible".
- sigmoid of near-const inputs (width of the sigma output interval < ~2e-6)
  is replaced by its midpoint constant; the xor against a constant xs
  factors into a quadratic in ys: 1 - (a1+b1*ys)(a2+b2*ys).
- rot63(x) = 2x - [x>=0.5] exactly, up to a dropped x*2^-63 term that only
  survives rounding when frac(2x) == 0 (measure-zero, magnitude 5e-20).
- soft_xor / soft_add results carry deferred affine (scale, bias): biases
  fold into ACT sigmoid bias constants and stt scalars for free.
"""
import sys
sys.path.insert(0, "/opt/trn_rl_repo")
import math
import numpy as np
from concourse import bass, mybir
from concourse.tile import TileContext
from concourse.bass_primitives_rust import SemaphoreHandle
from concourse.bass import _bass_rust

A = mybir.AluOpType
F = mybir.ActivationFunctionType
DT = mybir.dt.float32

# ---------------------------------------------------------------- geometry
P = 128
FD = 980
BLOCK_ROWS = P * FD
BLOCKS = 2
CORE_ROWS = BLOCK_ROWS * BLOCKS
N_CORES = 8
TOTAL_ROWS = 2_000_000
PAD_ROWS = CORE_ROWS * N_CORES

ROUNDS = 10
G_SCHEDULE = [
    (0, 4, 8, 12, 0, 1), (1, 5, 9, 13, 2, 3), (2, 6, 10, 14, 4, 5), (3, 7, 11, 15, 6, 7),
    (0, 5, 10, 15, 8, 9), (1, 6, 11, 12, 10, 11), (2, 7, 8, 13, 12, 13), (3, 4, 9, 14, 14, 15),
]
_IV_INTS = [7640891576956012808, 13503953896175478587, 4354685564936845355,
            11912009170470909681, 5840696475078001361, 11170449401992604703,
            2270897969802886507, 6620516959819538809]
IV = (np.asarray(_IV_INTS, dtype=np.float32) / np.float32(2.0**64)).astype(np.float32)

EPSB = 3e-6            # interval widening for hw sigmoid inexactness
DROP_ADD_TOL = 2.5e-7  # drop soft_add addends with |value| below this
XS_CONST_TOL = 3e-6    # sigma-output interval width below which xs is const

POOL_TT = True         # allow Pool engine for tensor_tensor add/sub/mult
POOL_HANDICAP = 1.0    # decision-time multiplier on Pool cost (Pool ops add
                       # ~0.9us latency to serial chains; offload only under
                       # real DVE pressure)
DROP_D16 = True        # drop the 1.5e-5 rot16 addend into c (kills the whole
                       # runtime d-lineage; error budget ~1e-4 vs 2e-2 gate)
DROP_D16_TOL = 2e-5


def configure(fd=980, blocks=2):
    global FD, BLOCK_ROWS, BLOCKS, CORE_ROWS, PAD_ROWS
    FD = fd
    BLOCKS = blocks
    BLOCK_ROWS = P * FD
    CORE_ROWS = BLOCK_ROWS * BLOCKS
    PAD_ROWS = CORE_ROWS * N_CORES


def f32(x):
    return np.float32(x)


def sig64(z):
    z = float(z)
    if z >= 0:
        return 1.0 / (1.0 + math.exp(-z))
    e = math.exp(z)
    return e / (1.0 + e)


def sigc(z):
    """f32-rounded sigmoid of f64 arg."""
    return float(np.float32(sig64(z)))


# cost estimates (per-op ns at current FD), calibrated against TimelineSim
# with synthetic back-to-back op streams (includes per-op sync overheads)
def ns_tt():
    return (FD + 58) / 0.96 + 134

def ns_ts():
    return (FD / 2 + 58) / 0.96 + 174

def ns_act():
    return (FD + 352) / 1.2

def ns_pool_tt():
    return (FD / 1.2) / 0.42 + 170


class Val:
    """true_value = scale * ap[...] + bias, or a build-time const.
    lo/hi bound the TRUE value."""
    __slots__ = ("const", "ap", "scale", "bias", "lo", "hi")

    def __init__(self, const=None, ap=None, scale=1.0, bias=0.0, lo=None, hi=None):
        self.const = const
        self.ap = ap
        self.scale = float(scale)
        self.bias = float(bias)
        if const is not None:
            self.lo = self.hi = float(const)
        else:
            assert lo is not None and hi is not None, "tensor Val needs bounds"
            self.lo = float(lo)
            self.hi = float(hi)

    @property
    def is_const(self):
        return self.const is not None

    def absmax(self):
        return max(abs(self.lo), abs(self.hi))


def sadd_const(a, b):
    """f32-faithful soft_add of two consts."""
    s = f32(f32(a) + f32(b))
    z = f32(f32(10.0) * f32(s - f32(1.0)))
    w = f32(sig64(float(z)))
    return float(f32(s - w))


def sxor_const(x, y):
    xs = f32(sig64(float(f32(f32(10.0) * f32(f32(x) - f32(0.5))))))
    ys = f32(sig64(float(f32(f32(10.0) * f32(f32(y) - f32(0.5))))))
    t1 = f32(xs * f32(f32(1.0) - ys))
    t2 = f32(f32(f32(1.0) - xs) * ys)
    r = f32(f32(t1 + t2) - f32(t1 * t2))
    return float(min(max(float(r), 0.0), 1.0))


def rot63_const(c):
    c = f32(c)
    m = f32(math.floor(float(c) * 2.0))  # floor(2c) for c in [0,1)
    sl = f32(f32(c * f32(2.0**65)) - f32(m * f32(2.0**64)))
    s = f32(f32(c * f32(2.0)) + sl)
    # mod(s, 2^64) with s < 2^64 is identity
    return float(f32(s / f32(2.0**64)))


def half_ulp_floor(x):
    """smallest half-ulp among f32 values >= x (x > 0)."""
    assert x > 0
    _, e = math.frexp(x)  # x = m * 2^e, m in [0.5, 1)
    return 2.0 ** (e - 25)


class Prog:
    def __init__(self):
        self.nc = bass.Bass("TRN2")
        self.est = {"dve": 0.0, "act": 0.0, "pool": 0.0}
        self._lane = None
        self._lane_id = 0
        self._bias_tiles = {}
        self._bias_pool = None
        self._bias_count = 0

    # ---------------- lane machinery (same as baseline)
    def _run(self, fn):
        if self._lane is not None:
            self._lane.append(fn)
        else:
            fn()

    def begin_lane(self, lane, lane_id=0):
        self._lane = lane
        self._lane_id = lane_id

    def end_lane(self):
        self._lane = None

    def merge_lanes(self, lanes):
        lanes = [list(l) for l in lanes if l]
        while lanes:
            nxt = []
            for l in lanes:
                l.pop(0)()
                if l:
                    nxt.append(l)
            lanes = nxt

    # ---------------- scratch
    def scr(self):
        cell = {}
        tag = f"scr{self._lane_id or 0}"
        def get():
            if "t" not in cell:
                cell["t"] = self.scr_pool.tile([P, FD], DT, tag=tag, name=tag, bufs=4)
            return cell["t"][:]
        return get

    # ---------------- bias const tiles for ACT sigmoid
    def bias_ap(self, value):
        v = float(np.float32(value))
        if v not in self._bias_tiles:
            t = self._bias_pool.tile([P, 1], DT, tag=f"bias{self._bias_count}",
                                     name=f"bias{self._bias_count}")
            self._bias_count += 1
            # eager emit (before any lane-deferred consumer is flushed)
            self.nc.vector.memset(t[:], v)
            self.est["dve"] += 65.0
            self._bias_tiles[v] = t
        t = self._bias_tiles[v]
        return lambda: t[:]

    # ---------------- balanced emitters
    def tt(self, out, a, b, op):
        """tensor_tensor; Pool-eligible for add/sub/mult."""
        pool_ok = POOL_TT and op in (A.add, A.subtract, A.mult)
        cd, cp = ns_tt(), ns_pool_tt()
        if pool_ok and self.est["pool"] + cp < POOL_HANDICAP * (self.est["dve"] + cd):
            self.est["pool"] += cp
            self._run(lambda: self.nc.gpsimd.tensor_tensor(out(), a(), b(), op=op))
        else:
            self.est["dve"] += cd
            self._run(lambda: self.nc.vector.tensor_tensor(out(), a(), b(), op=op))

    def stt(self, out, in0, scalar, in1, op0, op1, rev0=False):
        def fn():
            i = self.nc.vector.scalar_tensor_tensor(
                out(), in0(), float(scalar), in1(), op0=op0, op1=op1)
            if rev0:
                i.ins.reverse0 = True
        self.est["dve"] += ns_tt()
        self._run(fn)

    def ts_cmp(self, out, in0, thresh, op0):
        """compare tensor_scalar — DVE only."""
        self.est["dve"] += ns_ts()
        self._run(lambda: self.nc.vector.tensor_scalar(out(), in0(), float(thresh), None, op0=op0))

    def affine(self, out, in0, scale, bias, strided=False):
        """out = scale*in0 + bias on DVE-ts or ACT-copy, balancer's choice."""
        scale = float(np.float32(scale))
        bias = float(np.float32(bias))
        cd = ns_tt() if strided else ns_ts()
        ca = ns_act()
        if self.est["act"] + ca < self.est["dve"] + cd:
            self.est["act"] += ca
            self._run(lambda: self.nc.scalar.activation(out(), in0(), F.Copy,
                                                        bias=bias, scale=scale))
        else:
            self.est["dve"] += cd
            if bias == 0.0:
                self._run(lambda: self.nc.vector.tensor_scalar(out(), in0(), scale, None, op0=A.mult))
            else:
                self._run(lambda: self.nc.vector.tensor_scalar(out(), in0(), scale, bias,
                                                               op0=A.mult, op1=A.add))

    def affine1m(self, out, in0):
        """out = 1 - in0."""
        cd, ca = ns_ts(), ns_act()
        if self.est["act"] + ca < self.est["dve"] + cd:
            self.est["act"] += ca
            self._run(lambda: self.nc.scalar.activation(out(), in0(), F.Copy,
                                                        bias=1.0, scale=-1.0))
        else:
            self.est["dve"] += cd
            def fn():
                i = self.nc.vector.tensor_scalar(out(), in0(), 1.0, None, op0=A.subtract)
                i.ins.reverse0 = True
            self._run(fn)

    def sigmoid(self, out, in_ap, scale, biasval):
        """out = sigmoid(scale * in + biasval); biasval via const tile."""
        b = self.bias_ap(biasval)
        s = float(np.float32(scale))
        self.est["act"] += ns_act()
        self._run(lambda: self.nc.scalar.activation(out(), in_ap(), F.Sigmoid,
                                                    bias=b(), scale=s))

    # ---------------- interval helpers
    def sig_interval(self, V):
        """interval of sigmoid(10*(V-0.5)) over V's bounds (widened)."""
        if V.is_const:
            x = sigc(f32(f32(10.0) * f32(f32(V.const) - f32(0.5))))
            return (x, x)
        lo = sig64(10.0 * (V.lo - EPSB - 0.5)) - 1e-7
        hi = sig64(10.0 * (V.hi + EPSB - 0.5)) + 1e-7
        return (lo, hi)

    # ---------------- soft primitives
    def soft_add(self, dst_slot, Av, Bv, dst_ap=None):
        if Av.is_const and Bv.is_const:
            return Val(const=sadd_const(Av.const, Bv.const))

        # skip/drop tiny addend (exactness or tolerance based)
        for X, Y in ((Av, Bv), (Bv, Av)):
            if Y.is_const or Y.absmax() > 2e-5:
                continue
            ymax = Y.absmax()
            exact_ok = (X.lo > 1e-30) and (ymax < 0.99 * half_ulp_floor(X.lo))
            drop_tol = DROP_D16_TOL if DROP_D16 else DROP_ADD_TOL
            if ymax <= drop_tol or exact_ok:
                if X.is_const:
                    return Val(const=sadd_const(X.const, 0.0))
                return self._sadd_finish(dst_slot, X.ap, X.scale, X.bias,
                                         Av, Bv, dst_ap)

        if Av.is_const or Bv.is_const:
            c, T = (Av.const, Bv) if Av.is_const else (Bv.const, Av)
            return self._sadd_finish(dst_slot, T.ap, T.scale, T.bias + c,
                                     Av, Bv, dst_ap)

        # combine two tensors
        sa, sb = Av.scale, Bv.scale
        h = self.scr()
        if sa == sb:
            self.tt(h, Av.ap, Bv.ap, A.add)
            ss = sa
        elif sa == -sb:
            self.tt(h, Av.ap, Bv.ap, A.subtract)
            ss = sa
        else:
            # keep |ratio| <= 1: fold the smaller-scale operand in scaled form
            if abs(sa) <= abs(sb):
                self.stt(h, Av.ap, sa / sb, Bv.ap, A.mult, A.add)
                ss = sb
            else:
                self.stt(h, Bv.ap, sb / sa, Av.ap, A.mult, A.add)
                ss = sa
        return self._sadd_finish(dst_slot, h, ss, Av.bias + Bv.bias, Av, Bv, dst_ap)

    def _sadd_finish(self, dst_slot, h, ss, beta, Av, Bv, dst_ap):
        # bounds of true output
        slo, shi = Av.lo + Bv.lo, Av.hi + Bv.hi
        lo = slo - sig64(10.0 * (shi - 1.0)) - EPSB
        hi = shi - sig64(10.0 * (slo - 1.0)) + EPSB
        w = self.scr()
        self.sigmoid(w, h, 10.0 * ss, 10.0 * beta - 10.0)
        dst = dst_ap if dst_ap is not None else self.v_aps[dst_slot]
        if ss == 1.0:
            self.tt(dst, h, w, A.subtract)
        else:
            self.stt(dst, h, ss, w, A.mult, A.subtract)
        return Val(ap=dst, scale=1.0, bias=beta, lo=lo, hi=hi)

    def soft_xor(self, dst_slot, Xv, Yv, dst_ap=None, materialize=False):
        if Xv.is_const and Yv.is_const:
            return Val(const=sxor_const(Xv.const, Yv.const))

        xi = self.sig_interval(Xv)
        yi = self.sig_interval(Yv)
        x_constish = Xv.is_const or (xi[1] - xi[0] <= XS_CONST_TOL)
        y_constish = Yv.is_const or (yi[1] - yi[0] <= XS_CONST_TOL)

        if x_constish and y_constish:
            # both sides' sigmoids constant: output is a build-time const
            xbar, ybar = (xi[0] + xi[1]) / 2, (yi[0] + yi[1]) / 2
            t1 = xbar * (1.0 - ybar)
            t2 = (1.0 - xbar) * ybar
            return Val(const=float(f32(t1 + t2 - t1 * t2)))
        if x_constish:
            return self._sxor_const_side(dst_slot, xi, Yv, yi, dst_ap, materialize)
        if y_constish:
            return self._sxor_const_side(dst_slot, yi, Xv, xi, dst_ap, materialize)
        return self._sxor_full(dst_slot, Xv, xi, Yv, yi, dst_ap, materialize)

    def _xor_bounds(self, xi, yi):
        corners = [(a, b) for a in xi for b in yi]
        vals = [a + b - 2 * a * b for a, b in corners]
        m, M = min(vals), max(vals)
        lo = max(0.0, m - m * m / 4.0) - EPSB
        hi = min(M, 1.0) + EPSB
        return lo, hi

    def _sxor_const_side(self, dst_slot, xi, Yv, yi, dst_ap, materialize):
        """xs is effectively const xbar; out = 1 - (a1+b1*ys)(a2+b2*ys)."""
        # canonicalize tiny-class midpoints so emitted constants (and ACT
        # bias tiles) repeat across rounds instead of tracking each
        # instance's slightly-different interval
        if xi[1] - xi[0] > 0:
            if xi[1] <= sig64(10 * (3e-10 - 0.5)) + 1e-6:
                xbar = sigc(10.0 * (2.0**-33 - 0.5))
            elif xi[1] <= sig64(10 * (3e-7 - 0.5)) + 1e-6:
                xbar = sigc(10.0 * (2.0**-25 - 0.5))
            elif xi[1] <= sig64(10 * (3e-5 - 0.5)) + 1e-6:
                xbar = sigc(10.0 * (2.0**-17 - 0.5))
            else:
                xbar = (xi[0] + xi[1]) / 2
        else:
            xbar = (xi[0] + xi[1]) / 2
        a1, b1 = 1.0 - xbar, xbar
        a2, b2 = 1.0, -(1.0 - xbar)
        c0 = a1 * a2
        c1 = a1 * b2 + b1 * a2
        c2 = b1 * b2
        lo, hi = self._xor_bounds(xi, yi)
        ys = self.scr()
        self.sigmoid(ys, Yv.ap, 10.0 * Yv.scale, 10.0 * Yv.bias - 5.0)
        dst = dst_ap if dst_ap is not None else self.v_aps[dst_slot]
        # Square-ACT variant: q = c2*ys^2 + c1*ys = b^2 - (s*ys + bq)^2 with
        # s = sqrt(-c2), bq = -c1/(2s); out = (1-c0-bq^2) + S. Pure-ACT (2 ops)
        # vs sigma + affine + tt. Pick by projected engine load.
        sq_s = math.sqrt(-c2)
        sq_b = -c1 / (2.0 * sq_s)
        ca, cd_extra = ns_act(), ns_ts() + ns_tt()
        use_sq = (not materialize) and (
            self.est["act"] + ca <= min(self.est["dve"] + cd_extra,
                                        self.est["pool"] + ns_pool_tt() + ns_ts()))
        if use_sq:
            b_ap = self.bias_ap(sq_b)
            self.est["act"] += ca
            self._run(lambda: self.nc.scalar.activation(
                dst(), ys(), F.Square, bias=b_ap(), scale=float(np.float32(sq_s))))
            return Val(ap=dst, scale=1.0, bias=1.0 - c0 - sq_b * sq_b, lo=lo, hi=hi)
        t = self.scr()
        self.affine(t, ys, c2, c1)
        if materialize:
            q = self.scr()
            self.tt(q, t, ys, A.mult)
            self.affine(dst, q, -1.0, 1.0 - c0, strided=True)
            return Val(ap=dst, scale=1.0, bias=0.0, lo=lo, hi=hi)
        self.tt(dst, t, ys, A.mult)
        return Val(ap=dst, scale=-1.0, bias=1.0 - c0, lo=lo, hi=hi)

    def _sxor_full(self, dst_slot, Xv, xi, Yv, yi, dst_ap, materialize):
        lo, hi = self._xor_bounds(xi, yi)
        xs = self.scr()
        ys = self.scr()
        t1 = self.scr()
        self.sigmoid(xs, Xv.ap, 10.0 * Xv.scale, 10.0 * Xv.bias - 5.0)
        self.sigmoid(ys, Yv.ap, 10.0 * Yv.scale, 10.0 * Yv.bias - 5.0)
        self.stt(t1, ys, 1.0, xs, A.subtract, A.mult, rev0=True)   # (1-ys)*xs
        self.stt(xs, xs, 1.0, ys, A.subtract, A.mult, rev0=True)   # xs <- t2=(1-xs)*ys
        self.affine1m(ys, xs)                                      # ys <- 1-t2
        dst = dst_ap if dst_ap is not None else self.v_aps[dst_slot]
        if materialize:
            self.stt(t1, t1, 1.0, ys, A.subtract, A.mult, rev0=True)  # (1-t1)(1-t2)
            self.affine(dst, t1, -1.0, 1.0, strided=True)
            return Val(ap=dst, scale=1.0, bias=0.0, lo=lo, hi=hi)
        self.stt(dst, t1, 1.0, ys, A.subtract, A.mult, rev0=True)
        return Val(ap=dst, scale=-1.0, bias=1.0, lo=lo, hi=hi)

    def soft_xor_dead(self, Xv, Yv):
        """soft_xor whose RUNTIME value is never consumed (only its bounds
        feed later const-folds). Emits nothing; returns a phantom Val whose
        ap raises if ever dereferenced."""
        if Xv.is_const and Yv.is_const:
            return Val(const=sxor_const(Xv.const, Yv.const))
        xi = self.sig_interval(Xv)
        yi = self.sig_interval(Yv)
        lo, hi = self._xor_bounds(xi, yi)
        def phantom():
            raise AssertionError("phantom (value-dead) soft_xor output was dereferenced")
        return Val(ap=phantom, scale=-1.0, bias=1.0, lo=lo, hi=hi)

    def rotate(self, slot, n, V):
        if V.is_const:
            if n in (16, 24, 32):
                return Val(const=float(f32(V.const)) * 2.0 ** (-n))
            assert n == 63
            return Val(const=rot63_const(V.const))
        if n in (16, 24, 32):
            need = {16: 2.0**-25, 24: 2.0**-17, 32: 2.0**-9}[n]
            assert V.lo >= need, f"rot{n} scale-defer needs lo>={need}, got {V.lo}"
            k = 2.0 ** (-n)
            return Val(ap=V.ap, scale=V.scale * k, bias=V.bias * k,
                       lo=V.lo * k, hi=V.hi * k)
        assert n == 63
        m = self.scr()
        dst = self.v_aps[slot]
        s, b = V.scale, V.bias
        # mask = [X >= 0.5] with X = s*u + b
        if s < 0:
            self.ts_cmp(m, V.ap, (b - 0.5) / (-s), A.is_le)
        else:
            self.ts_cmp(m, V.ap, (0.5 - b) / s, A.is_ge)
        # r = 2X - m = (2s)*u - m, bias 2b deferred
        self.stt(dst, V.ap, 2.0 * s, m, A.mult, A.subtract)
        return Val(ap=dst, scale=1.0, bias=2.0 * b, lo=-EPSB, hi=1.0 + EPSB)

    # ---------------- G function
    def G(self, vals, a, b, c, d, xi, yi):
        mx = Val(ap=self.m_aps[xi], lo=0.0, hi=1.0)
        my = Val(ap=self.m_aps[yi], lo=0.0, hi=1.0)
        vals[a] = self.soft_add(a, vals[a], vals[b])
        vals[a] = self.soft_add(a, vals[a], mx)
        # #3's output only survives rot32 (sub-half-ULP everywhere) -> its
        # runtime value is dead; bounds still feed #5's skip and #10's consts
        vals[d] = self.soft_xor_dead(vals[d], vals[a])
        vals[d] = self.rotate(d, 32, vals[d])
        vals[c] = self.soft_add(c, vals[c], vals[d])
        # #6's output only survives rot24: dropped by #8 (<=2.5e-7) and
        # const-folded by #13 -> value-dead as well
        vals[b] = self.soft_xor_dead(vals[b], vals[c])
        vals[b] = self.rotate(b, 24, vals[b])
        vals[a] = self.soft_add(a, vals[a], vals[b])
        vals[a] = self.soft_add(a, vals[a], my)
        if DROP_D16:
            # with the d16 addend dropped in #12, #10's runtime value is
            # dead as well (its other consumers const-fold it)
            vals[d] = self.soft_xor_dead(vals[d], vals[a])
        else:
            vals[d] = self.soft_xor(d, vals[d], vals[a])
        vals[d] = self.rotate(d, 16, vals[d])
        vals[c] = self.soft_add(c, vals[c], vals[d])
        vals[b] = self.soft_xor(b, vals[b], vals[c])
        vals[b] = self.rotate(b, 63, vals[b])

    # ---------------- whole program
    def build(self, scr_bufs=10):
        nc = self.nc
        self.msg = nc.declare_dram_parameter("message", [CORE_ROWS, 16], DT, isOutput=False)
        self.out = nc.declare_dram_parameter("out", [CORE_ROWS, 8], DT, isOutput=True)
        with TileContext(nc) as tc:
            with (
                tc.tile_pool(name="persist", bufs=1) as pp,
                tc.tile_pool(name="scrp", bufs=scr_bufs) as sp,
            ):
                self.scr_pool = sp
                self._bias_pool = pp
                m_tile = pp.tile([P, 16 * FD], DT, tag="m_stage", name="m_stage")
                out_tile = pp.tile([P, 8 * FD], DT, tag="out_stage", name="out_stage")
                v_tiles = [pp.tile([P, FD], DT, tag=f"v{j}", name=f"v{j}") for j in range(4)]
                self.v_aps = [(lambda jj=j: v_tiles[jj][:]) for j in range(4)] + [None] * 12

                for blk in range(BLOCKS):
                    r0 = blk * BLOCK_ROWS
                    self.m_aps = [(lambda jj=j: m_tile[:][:, jj::16]) for j in range(16)]
                    in_ap = self.msg[r0:r0 + BLOCK_ROWS, :].rearrange("(p f) w -> p (f w)", p=P)
                    nc.sync.dma_start(out=m_tile[:], in_=in_ap)
                    state = [Val(const=float(IV[j])) for j in range(8)]
                    # With b/c/d-words all const, the four a-word chains are
                    # fully independent across ALL rounds: emit each word's
                    # whole-block chain into one mega-lane (max scheduler slack)
                    block_lanes = [[] for _ in range(4)]
                    for rnd in range(ROUNDS):
                        vals = {}
                        for j in range(8):
                            vals[j] = state[j]
                            vals[8 + j] = Val(const=float(IV[j]))
                        for grp in (G_SCHEDULE[:4], G_SCHEDULE[4:]):
                            for li, (a, b, c, d, gx, gy) in enumerate(grp):
                                self.begin_lane(block_lanes[a], a)
                                self.G(vals, a, b, c, d, gx, gy)
                                self.end_lane()
                        last = rnd == ROUNDS - 1
                        new_state = [None] * 8
                        for j in range(8):
                            self.begin_lane(block_lanes[j % 4], j % 4)
                            if last:
                                dst = (lambda jj=j: out_tile[:][:, jj::8])
                                new_state[j] = self.soft_xor(
                                    None, vals[j], vals[8 + j], dst_ap=dst,
                                    materialize=True)
                                if new_state[j].is_const:
                                    cv = float(np.float32(new_state[j].const))
                                    self._run(lambda dd=dst, vv=cv:
                                              self.nc.vector.memset(dd(), vv))
                                    self.est["dve"] += ns_tt()
                            else:
                                new_state[j] = self.soft_xor(j, vals[j], vals[8 + j])
                            self.end_lane()
                        state = new_state
                    self.merge_lanes(block_lanes)
                    out_ap = self.out[r0:r0 + BLOCK_ROWS, :].rearrange("(p f) w -> p (f w)", p=P)
                    nc.sync.dma_start(out=out_ap, in_=out_tile[:])
        hoist_excess_waits(nc)
        return nc


def hoist_excess_waits(nc, max_waits=1):
    """Walrus can't encode >~2 sync waits per instruction; move excess into
    standalone NoOps (1 wait each) right before the instruction."""
    n_hoisted = 0
    for fu in nc.m.functions:
        for blk in fu.blocks:
            need = False
            for inst in blk.instructions:
                si = inst.sync_info
                if si is not None and len(si.on_wait) > max_waits:
                    need = True
                    break
            if not need:
                continue
            newl = []
            for inst in blk.instructions:
                si = inst.sync_info
                if si is not None and len(si.on_wait) > max_waits:
                    conds = list(si.on_wait)
                    keep = conds[-max_waits:]
                    for cnd in conds[:-max_waits]:
                        nop = mybir.InstNoOp(
                            name=nc.get_next_instruction_name(), ins=[], outs=[])
                        nop.engine = inst.engine
                        _bass_rust.wait_op(
                            nop, SemaphoreHandle(cnd.ant_name, cnd.id),
                            cnd.wait_value, "sem-ge", False)
                        newl.append(nop)
                        n_hoisted += 1
                    inst.sync_info = mybir.SyncInfo(on_wait=keep, on_update=list(si.on_update))
                newl.append(inst)
            blk.instructions = newl
    return n_hoisted


def build_program():
    p = Prog()
    nc = p.build()
    return nc, p


# ----------------------------------------------------------------- entry
_cache = {}


def _get_nc():
    if "nc" not in _cache:
        _cache["nc"] = build_program()[0]
    return _cache["nc"]


def kernel(message, _trace=False):
    """Full (2000000, 16) f32 in -> (2000000, 8) f32 out, 8-core data parallel."""
    from concourse.bass_utils import run_bass_kernel_spmd
    msg = np.ascontiguousarray(np.asarray(message, dtype=np.float32))
    nc = _get_nc()
    pad = PAD_ROWS - msg.shape[0]
    msgp = np.concatenate([msg, np.zeros((pad, 16), np.float32)]) if pad > 0 else msg
    shards = msgp.reshape(N_CORES, CORE_ROWS, 16)
    in_maps = [{"message": shards[i]} for i in range(N_CORES)]
    kw = dict(trace=True) if _trace else {}
    res = run_bass_kernel_spmd(nc, in_maps, core_ids=list(range(N_CORES)), **kw)
    out = np.concatenate([res.results[i]["out"] for i in range(N_CORES)], axis=0)
    if _trace:
        _cache["last_result"] = res
    return out[: msg.shape[0]]


# revision 33
# speedup vs baseline: 5.4865x; 1.0051x over previous
"""Blake2 soft-cipher Bass kernel for Trainium2 (8 NeuronCores, data parallel).

v2: affine-deferred values + interval-tracked tiny-value elimination +
3-engine (DVE/ACT/Pool) load balancing.

Key numerical facts (all verified against f32 reference semantics):
- rot16/24/32 of any soft_xor output are exact scales by 2^-n (the wrapped
  fraction is identically zero because xor outputs are >= ~0.0132 > 2^-9).
- rot32-scaled values (<= 2^-32) are sub-half-ULP against every downstream
  addend (c-words >= 0.0131 => half-ulp >= 2^-31 > 2^-32), and shift sigmoid
  inputs by < 0.03 ulp: both uses collapse to "the tiny operand is invisible".
- sigmoid of near-const inputs (width of the sigma output interval < ~2e-6)
  is replaced by its midpoint constant; the xor against a constant xs
  factors into a quadratic in ys: 1 - (a1+b1*ys)(a2+b2*ys).
- rot63(x) = 2x - [x>=0.5] exactly, up to a dropped x*2^-63 term that only
  survives rounding when frac(2x) == 0 (measure-zero, magnitude 5e-20).
- soft_xor / soft_add results carry deferred affine (scale, bias): biases
  fold into ACT sigmoid bias constants and stt scalars for free.
"""
import sys
sys.path.insert(0, "/opt/trn_rl_repo")
import math
import numpy as np
from concourse import bass, mybir
from concourse.tile import TileContext
from concourse.bass_primitives_rust import SemaphoreHandle
from concourse.bass import _bass_rust

A = mybir.AluOpType
F = mybir.ActivationFunctionType
DT = mybir.dt.float32

# ---------------------------------------------------------------- geometry
P = 128
FD = 980
BLOCK_ROWS = P * FD
BLOCKS = 2
CORE_ROWS = BLOCK_ROWS * BLOCKS
N_CORES = 8
TOTAL_ROWS = 2_000_000
PAD_ROWS = CORE_ROWS * N_CORES

ROUNDS = 10
G_SCHEDULE = [
    (0, 4, 8, 12, 0, 1), (1, 5, 9, 13, 2, 3), (2, 6, 10, 14, 4, 5), (3, 7, 11, 15, 6, 7),
    (0, 5, 10, 15, 8, 9), (1, 6, 11, 12, 10, 11), (2, 7, 8, 13, 12, 13), (3, 4, 9, 14, 14, 15),
]
_IV_INTS = [7640891576956012808, 13503953896175478587, 4354685564936845355,
            11912009170470909681, 5840696475078001361, 11170449401992604703,
            2270897969802886507, 6620516959819538809]
IV = (np.asarray(_IV_INTS, dtype=np.float32) / np.float32(2.0**64)).astype(np.float32)

EPSB = 3e-6            # interval widening for hw sigmoid inexactness
DROP_ADD_TOL = 2.5e-7  # drop soft_add addends with |value| below this
XS_CONST_TOL = 3e-6    # sigma-output interval width below which xs is const

POOL_TT = True         # allow Pool engine for tensor_tensor add/sub/mult
POOL_HANDICAP = 1.0    # decision-time multiplier on Pool cost (Pool ops add
                       # ~0.9us latency to serial chains; offload only under
                       # real DVE pressure)
DROP_D16 = True        # drop the 1.5e-5 rot16 addend into c (kills the whole
                       # runtime d-lineage; error budget ~1e-4 vs 2e-2 gate)
DROP_D16_TOL = 2e-5


def configure(fd=980, blocks=2):
    global FD, BLOCK_ROWS, BLOCKS, CORE_ROWS, PAD_ROWS
    FD = fd
    BLOCKS = blocks
    BLOCK_ROWS = P * FD
    CORE_ROWS = BLOCK_ROWS * BLOCKS
    PAD_ROWS = CORE_ROWS * N_CORES


def f32(x):
    return np.float32(x)


def sig64(z):
    z = float(z)
    if z >= 0:
        return 1.0 / (1.0 + math.exp(-z))
    e = math.exp(z)
    return e / (1.0 + e)


def sigc(z):
    """f32-rounded sigmoid of f64 arg."""
    return float(np.float32(sig64(z)))


# cost estimates (per-op ns at current FD), calibrated against TimelineSim
# with synthetic back-to-back op streams (includes per-op sync overheads)
def ns_tt():
    return (FD + 58) / 0.96 + 134

def ns_ts():
    return (FD / 2 + 58) / 0.96 + 174

def ns_act():
    return (FD + 352) / 1.2

def ns_pool_tt():
    return (FD / 1.2) / 0.42 + 170


class Val:
    """true_value = scale * ap[...] + bias, or a build-time const.
    lo/hi bound the TRUE value."""
    __slots__ = ("const", "ap", "scale", "bias", "lo", "hi")

    def __init__(self, const=None, ap=None, scale=1.0, bias=0.0, lo=None, hi=None):
        self.const = const
        self.ap = ap
        self.scale = float(scale)
        self.bias = float(bias)
        if const is not None:
            self.lo = self.hi = float(const)
        else:
            assert lo is not None and hi is not None, "tensor Val needs bounds"
            self.lo = float(lo)
            self.hi = float(hi)

    @property
    def is_const(self):
        return self.const is not None

    def absmax(self):
        return max(abs(self.lo), abs(self.hi))


def sadd_const(a, b):
    """f32-faithful soft_add of two consts."""
    s = f32(f32(a) + f32(b))
    z = f32(f32(10.0) * f32(s - f32(1.0)))
    w = f32(sig64(float(z)))
    return float(f32(s - w))


def sxor_const(x, y):
    xs = f32(sig64(float(f32(f32(10.0) * f32(f32(x) - f32(0.5))))))
    ys = f32(sig64(float(f32(f32(10.0) * f32(f32(y) - f32(0.5))))))
    t1 = f32(xs * f32(f32(1.0) - ys))
    t2 = f32(f32(f32(1.0) - xs) * ys)
    r = f32(f32(t1 + t2) - f32(t1 * t2))
    return float(min(max(float(r), 0.0), 1.0))


def rot63_const(c):
    c = f32(c)
    m = f32(math.floor(float(c) * 2.0))  # floor(2c) for c in [0,1)
    sl = f32(f32(c * f32(2.0**65)) - f32(m * f32(2.0**64)))
    s = f32(f32(c * f32(2.0)) + sl)
    # mod(s, 2^64) with s < 2^64 is identity
    return float(f32(s / f32(2.0**64)))


def half_ulp_floor(x):
    """smallest half-ulp among f32 values >= x (x > 0)."""
    assert x > 0
    _, e = math.frexp(x)  # x = m * 2^e, m in [0.5, 1)
    return 2.0 ** (e - 25)


class Prog:
    def __init__(self):
        self.nc = bass.Bass("TRN2")
        self.est = {"dve": 0.0, "act": 0.0, "pool": 0.0}
        self._lane = None
        self._lane_id = 0
        self._bias_tiles = {}
        self._bias_pool = None
        self._bias_count = 0

    # ---------------- lane machinery (same as baseline)
    def _run(self, fn):
        if self._lane is not None:
            self._lane.append(fn)
        else:
            fn()

    def begin_lane(self, lane, lane_id=0):
        self._lane = lane
        self._lane_id = lane_id

    def end_lane(self):
        self._lane = None

    def merge_lanes(self, lanes):
        lanes = [list(l) for l in lanes if l]
        while lanes:
            nxt = []
            for l in lanes:
                l.pop(0)()
                if l:
                    nxt.append(l)
            lanes = nxt

    # ---------------- scratch
    def scr(self):
        cell = {}
        tag = f"scr{self._lane_id or 0}"
        def get():
            if "t" not in cell:
                cell["t"] = self.scr_pool.tile([P, FD], DT, tag=tag, name=tag, bufs=3)
            return cell["t"][:]
        return get

    # ---------------- bias const tiles for ACT sigmoid
    def bias_ap(self, value):
        v = float(np.float32(value))
        if v not in self._bias_tiles:
            t = self._bias_pool.tile([P, 1], DT, tag=f"bias{self._bias_count}",
                                     name=f"bias{self._bias_count}")
            self._bias_count += 1
            # eager emit (before any lane-deferred consumer is flushed)
            self.nc.vector.memset(t[:], v)
            self.est["dve"] += 65.0
            self._bias_tiles[v] = t
        t = self._bias_tiles[v]
        return lambda: t[:]

    # ---------------- balanced emitters
    def tt(self, out, a, b, op):
        """tensor_tensor; Pool-eligible for add/sub/mult."""
        pool_ok = POOL_TT and op in (A.add, A.subtract, A.mult)
        cd, cp = ns_tt(), ns_pool_tt()
        if pool_ok and self.est["pool"] + cp < POOL_HANDICAP * (self.est["dve"] + cd):
            self.est["pool"] += cp
            self._run(lambda: self.nc.gpsimd.tensor_tensor(out(), a(), b(), op=op))
        else:
            self.est["dve"] += cd
            self._run(lambda: self.nc.vector.tensor_tensor(out(), a(), b(), op=op))

    def stt(self, out, in0, scalar, in1, op0, op1, rev0=False):
        def fn():
            i = self.nc.vector.scalar_tensor_tensor(
                out(), in0(), float(scalar), in1(), op0=op0, op1=op1)
            if rev0:
                i.ins.reverse0 = True
        self.est["dve"] += ns_tt()
        self._run(fn)

    def ts_cmp(self, out, in0, thresh, op0):
        """compare tensor_scalar — DVE only."""
        self.est["dve"] += ns_ts()
        self._run(lambda: self.nc.vector.tensor_scalar(out(), in0(), float(thresh), None, op0=op0))

    def affine(self, out, in0, scale, bias, strided=False):
        """out = scale*in0 + bias on DVE-ts or ACT-copy, balancer's choice."""
        scale = float(np.float32(scale))
        bias = float(np.float32(bias))
        cd = ns_tt() if strided else ns_ts()
        ca = ns_act()
        if self.est["act"] + ca < self.est["dve"] + cd:
            self.est["act"] += ca
            self._run(lambda: self.nc.scalar.activation(out(), in0(), F.Copy,
                                                        bias=bias, scale=scale))
        else:
            self.est["dve"] += cd
            if bias == 0.0:
                self._run(lambda: self.nc.vector.tensor_scalar(out(), in0(), scale, None, op0=A.mult))
            else:
                self._run(lambda: self.nc.vector.tensor_scalar(out(), in0(), scale, bias,
                                                               op0=A.mult, op1=A.add))

    def affine1m(self, out, in0):
        """out = 1 - in0."""
        cd, ca = ns_ts(), ns_act()
        if self.est["act"] + ca < self.est["dve"] + cd:
            self.est["act"] += ca
            self._run(lambda: self.nc.scalar.activation(out(), in0(), F.Copy,
                                                        bias=1.0, scale=-1.0))
        else:
            self.est["dve"] += cd
            def fn():
                i = self.nc.vector.tensor_scalar(out(), in0(), 1.0, None, op0=A.subtract)
                i.ins.reverse0 = True
            self._run(fn)

    def sigmoid(self, out, in_ap, scale, biasval):
        """out = sigmoid(scale * in + biasval); biasval via const tile."""
        b = self.bias_ap(biasval)
        s = float(np.float32(scale))
        self.est["act"] += ns_act()
        self._run(lambda: self.nc.scalar.activation(out(), in_ap(), F.Sigmoid,
                                                    bias=b(), scale=s))

    # ---------------- interval helpers
    def sig_interval(self, V):
        """interval of sigmoid(10*(V-0.5)) over V's bounds (widened)."""
        if V.is_const:
            x = sigc(f32(f32(10.0) * f32(f32(V.const) - f32(0.5))))
            return (x, x)
        lo = sig64(10.0 * (V.lo - EPSB - 0.5)) - 1e-7
        hi = sig64(10.0 * (V.hi + EPSB - 0.5)) + 1e-7
        return (lo, hi)

    # ---------------- soft primitives
    def soft_add(self, dst_slot, Av, Bv, dst_ap=None):
        if Av.is_const and Bv.is_const:
            return Val(const=sadd_const(Av.const, Bv.const))

        # skip/drop tiny addend (exactness or tolerance based)
        for X, Y in ((Av, Bv), (Bv, Av)):
            if Y.is_const or Y.absmax() > 2e-5:
                continue
            ymax = Y.absmax()
            exact_ok = (X.lo > 1e-30) and (ymax < 0.99 * half_ulp_floor(X.lo))
            drop_tol = DROP_D16_TOL if DROP_D16 else DROP_ADD_TOL
            if ymax <= drop_tol or exact_ok:
                if X.is_const:
                    return Val(const=sadd_const(X.const, 0.0))
                return self._sadd_finish(dst_slot, X.ap, X.scale, X.bias,
                                         Av, Bv, dst_ap)

        if Av.is_const or Bv.is_const:
            c, T = (Av.const, Bv) if Av.is_const else (Bv.const, Av)
            return self._sadd_finish(dst_slot, T.ap, T.scale, T.bias + c,
                                     Av, Bv, dst_ap)

        # combine two tensors
        sa, sb = Av.scale, Bv.scale
        h = self.scr()
        if sa == sb:
            self.tt(h, Av.ap, Bv.ap, A.add)
            ss = sa
        elif sa == -sb:
            self.tt(h, Av.ap, Bv.ap, A.subtract)
            ss = sa
        else:
            # keep |ratio| <= 1: fold the smaller-scale operand in scaled form
            if abs(sa) <= abs(sb):
                self.stt(h, Av.ap, sa / sb, Bv.ap, A.mult, A.add)
                ss = sb
            else:
                self.stt(h, Bv.ap, sb / sa, Av.ap, A.mult, A.add)
                ss = sa
        return self._sadd_finish(dst_slot, h, ss, Av.bias + Bv.bias, Av, Bv, dst_ap)

    def _sadd_finish(self, dst_slot, h, ss, beta, Av, Bv, dst_ap):
        # bounds of true output
        slo, shi = Av.lo + Bv.lo, Av.hi + Bv.hi
        lo = slo - sig64(10.0 * (shi - 1.0)) - EPSB
        hi = shi - sig64(10.0 * (slo - 1.0)) + EPSB
        w = self.scr()
        self.sigmoid(w, h, 10.0 * ss, 10.0 * beta - 10.0)
        dst = dst_ap if dst_ap is not None else self.scr()
        if ss == 1.0:
            self.tt(dst, h, w, A.subtract)
        else:
            self.stt(dst, h, ss, w, A.mult, A.subtract)
        return Val(ap=dst, scale=1.0, bias=beta, lo=lo, hi=hi)

    def soft_xor(self, dst_slot, Xv, Yv, dst_ap=None, materialize=False):
        if Xv.is_const and Yv.is_const:
            return Val(const=sxor_const(Xv.const, Yv.const))

        xi = self.sig_interval(Xv)
        yi = self.sig_interval(Yv)
        x_constish = Xv.is_const or (xi[1] - xi[0] <= XS_CONST_TOL)
        y_constish = Yv.is_const or (yi[1] - yi[0] <= XS_CONST_TOL)

        if x_constish and y_constish:
            # both sides' sigmoids constant: output is a build-time const
            xbar, ybar = (xi[0] + xi[1]) / 2, (yi[0] + yi[1]) / 2
            t1 = xbar * (1.0 - ybar)
            t2 = (1.0 - xbar) * ybar
            return Val(const=float(f32(t1 + t2 - t1 * t2)))
        if x_constish:
            return self._sxor_const_side(dst_slot, xi, Yv, yi, dst_ap, materialize)
        if y_constish:
            return self._sxor_const_side(dst_slot, yi, Xv, xi, dst_ap, materialize)
        return self._sxor_full(dst_slot, Xv, xi, Yv, yi, dst_ap, materialize)

    def _xor_bounds(self, xi, yi):
        corners = [(a, b) for a in xi for b in yi]
        vals = [a + b - 2 * a * b for a, b in corners]
        m, M = min(vals), max(vals)
        lo = max(0.0, m - m * m / 4.0) - EPSB
        hi = min(M, 1.0) + EPSB
        return lo, hi

    def _sxor_const_side(self, dst_slot, xi, Yv, yi, dst_ap, materialize):
        """xs is effectively const xbar; out = 1 - (a1+b1*ys)(a2+b2*ys)."""
        # canonicalize tiny-class midpoints so emitted constants (and ACT
        # bias tiles) repeat across rounds instead of tracking each
        # instance's slightly-different interval
        if xi[1] - xi[0] > 0:
            if xi[1] <= sig64(10 * (3e-10 - 0.5)) + 1e-6:
                xbar = sigc(10.0 * (2.0**-33 - 0.5))
            elif xi[1] <= sig64(10 * (3e-7 - 0.5)) + 1e-6:
                xbar = sigc(10.0 * (2.0**-25 - 0.5))
            elif xi[1] <= sig64(10 * (3e-5 - 0.5)) + 1e-6:
                xbar = sigc(10.0 * (2.0**-17 - 0.5))
            else:
                xbar = (xi[0] + xi[1]) / 2
        else:
            xbar = (xi[0] + xi[1]) / 2
        a1, b1 = 1.0 - xbar, xbar
        a2, b2 = 1.0, -(1.0 - xbar)
        c0 = a1 * a2
        c1 = a1 * b2 + b1 * a2
        c2 = b1 * b2
        lo, hi = self._xor_bounds(xi, yi)
        ys = self.scr()
        self.sigmoid(ys, Yv.ap, 10.0 * Yv.scale, 10.0 * Yv.bias - 5.0)
        dst = dst_ap if dst_ap is not None else self.scr()
        # Square-ACT variant: q = c2*ys^2 + c1*ys = b^2 - (s*ys + bq)^2 with
        # s = sqrt(-c2), bq = -c1/(2s); out = (1-c0-bq^2) + S. Pure-ACT (2 ops)
        # vs sigma + affine + tt. Pick by projected engine load.
        sq_s = math.sqrt(-c2)
        sq_b = -c1 / (2.0 * sq_s)
        ca, cd_extra = ns_act(), ns_ts() + ns_tt()
        use_sq = (not materialize) and (
            self.est["act"] + ca <= min(self.est["dve"] + cd_extra,
                                        self.est["pool"] + ns_pool_tt() + ns_ts()))
        if use_sq:
            b_ap = self.bias_ap(sq_b)
            self.est["act"] += ca
            self._run(lambda: self.nc.scalar.activation(
                dst(), ys(), F.Square, bias=b_ap(), scale=float(np.float32(sq_s))))
            return Val(ap=dst, scale=1.0, bias=1.0 - c0 - sq_b * sq_b, lo=lo, hi=hi)
        t = self.scr()
        self.affine(t, ys, c2, c1)
        if materialize:
            q = self.scr()
            self.tt(q, t, ys, A.mult)
            self.affine(dst, q, -1.0, 1.0 - c0, strided=True)
            return Val(ap=dst, scale=1.0, bias=0.0, lo=lo, hi=hi)
        self.tt(dst, t, ys, A.mult)
        return Val(ap=dst, scale=-1.0, bias=1.0 - c0, lo=lo, hi=hi)

    def _sxor_full(self, dst_slot, Xv, xi, Yv, yi, dst_ap, materialize):
        lo, hi = self._xor_bounds(xi, yi)
        xs = self.scr()
        ys = self.scr()
        t1 = self.scr()
        self.sigmoid(xs, Xv.ap, 10.0 * Xv.scale, 10.0 * Xv.bias - 5.0)
        self.sigmoid(ys, Yv.ap, 10.0 * Yv.scale, 10.0 * Yv.bias - 5.0)
        self.stt(t1, ys, 1.0, xs, A.subtract, A.mult, rev0=True)   # (1-ys)*xs
        self.stt(xs, xs, 1.0, ys, A.subtract, A.mult, rev0=True)   # xs <- t2=(1-xs)*ys
        self.affine1m(ys, xs)                                      # ys <- 1-t2
        dst = dst_ap if dst_ap is not None else self.scr()
        if materialize:
            self.stt(t1, t1, 1.0, ys, A.subtract, A.mult, rev0=True)  # (1-t1)(1-t2)
            self.affine(dst, t1, -1.0, 1.0, strided=True)
            return Val(ap=dst, scale=1.0, bias=0.0, lo=lo, hi=hi)
        self.stt(dst, t1, 1.0, ys, A.subtract, A.mult, rev0=True)
        return Val(ap=dst, scale=-1.0, bias=1.0, lo=lo, hi=hi)

    def soft_xor_dead(self, Xv, Yv):
        """soft_xor whose RUNTIME value is never consumed (only its bounds
        feed later const-folds). Emits nothing; returns a phantom Val whose
        ap raises if ever dereferenced."""
        if Xv.is_const and Yv.is_const:
            return Val(const=sxor_const(Xv.const, Yv.const))
        xi = self.sig_interval(Xv)
        yi = self.sig_interval(Yv)
        lo, hi = self._xor_bounds(xi, yi)
        def phantom():
            raise AssertionError("phantom (value-dead) soft_xor output was dereferenced")
        return Val(ap=phantom, scale=-1.0, bias=1.0, lo=lo, hi=hi)

    def rotate(self, slot, n, V):
        if V.is_const:
            if n in (16, 24, 32):
                return Val(const=float(f32(V.const)) * 2.0 ** (-n))
            assert n == 63
            return Val(const=rot63_const(V.const))
        if n in (16, 24, 32):
            need = {16: 2.0**-25, 24: 2.0**-17, 32: 2.0**-9}[n]
            assert V.lo >= need, f"rot{n} scale-defer needs lo>={need}, got {V.lo}"
            k = 2.0 ** (-n)
            return Val(ap=V.ap, scale=V.scale * k, bias=V.bias * k,
                       lo=V.lo * k, hi=V.hi * k)
        assert n == 63
        m = self.scr()
        dst = self.scr()
        s, b = V.scale, V.bias
        # mask = [X >= 0.5] with X = s*u + b
        if s < 0:
            self.ts_cmp(m, V.ap, (b - 0.5) / (-s), A.is_le)
        else:
            self.ts_cmp(m, V.ap, (0.5 - b) / s, A.is_ge)
        # r = 2X - m = (2s)*u - m, bias 2b deferred
        self.stt(dst, V.ap, 2.0 * s, m, A.mult, A.subtract)
        return Val(ap=dst, scale=1.0, bias=2.0 * b, lo=-EPSB, hi=1.0 + EPSB)

    # ---------------- G function
    def G(self, vals, a, b, c, d, xi, yi):
        mx = Val(ap=self.m_aps[xi], lo=0.0, hi=1.0)
        my = Val(ap=self.m_aps[yi], lo=0.0, hi=1.0)
        vals[a] = self.soft_add(a, vals[a], vals[b])
        vals[a] = self.soft_add(a, vals[a], mx)
        # #3's output only survives rot32 (sub-half-ULP everywhere) -> its
        # runtime value is dead; bounds still feed #5's skip and #10's consts
        vals[d] = self.soft_xor_dead(vals[d], vals[a])
        vals[d] = self.rotate(d, 32, vals[d])
        vals[c] = self.soft_add(c, vals[c], vals[d])
        # #6's output only survives rot24: dropped by #8 (<=2.5e-7) and
        # const-folded by #13 -> value-dead as well
        vals[b] = self.soft_xor_dead(vals[b], vals[c])
        vals[b] = self.rotate(b, 24, vals[b])
        vals[a] = self.soft_add(a, vals[a], vals[b])
        vals[a] = self.soft_add(a, vals[a], my)
        if DROP_D16:
            # with the d16 addend dropped in #12, #10's runtime value is
            # dead as well (its other consumers const-fold it)
            vals[d] = self.soft_xor_dead(vals[d], vals[a])
        else:
            vals[d] = self.soft_xor(d, vals[d], vals[a])
        vals[d] = self.rotate(d, 16, vals[d])
        vals[c] = self.soft_add(c, vals[c], vals[d])
        vals[b] = self.soft_xor(b, vals[b], vals[c])
        vals[b] = self.rotate(b, 63, vals[b])

    # ---------------- whole program
    def build(self, scr_bufs=10):
        nc = self.nc
        self.msg = nc.declare_dram_parameter("message", [CORE_ROWS, 16], DT, isOutput=False)
        self.out = nc.declare_dram_parameter("out", [CORE_ROWS, 8], DT, isOutput=True)
        with TileContext(nc) as tc:
            with (
                tc.tile_pool(name="persist", bufs=1) as pp,
                tc.tile_pool(name="scrp", bufs=scr_bufs) as sp,
            ):
                self.scr_pool = sp
                self._bias_pool = pp
                m_tiles = [pp.tile([P, 16 * FD], DT, tag=f"m_stage{i}", name=f"m_stage{i}")
                           for i in range(2)]
                out_tile = pp.tile([P, 8 * FD], DT, tag="out_stage", name="out_stage")
                self.v_aps = [None] * 16

                for blk in range(BLOCKS):
                    r0 = blk * BLOCK_ROWS
                    m_tile = m_tiles[blk % 2]
                    self.m_aps = [(lambda jj=j, mt=m_tile: mt[:][:, jj::16])
                                  for j in range(16)]
                    in_ap = self.msg[r0:r0 + BLOCK_ROWS, :].rearrange("(p f) w -> p (f w)", p=P)
                    nc.sync.dma_start(out=m_tile[:], in_=in_ap)
                    state = [Val(const=float(IV[j])) for j in range(8)]
                    # With b/c/d-words all const, the four a-word chains are
                    # fully independent across ALL rounds: emit each word's
                    # whole-block chain into one mega-lane (max scheduler slack)
                    block_lanes = [[] for _ in range(4)]
                    for rnd in range(ROUNDS):
                        vals = {}
                        for j in range(8):
                            vals[j] = state[j]
                            vals[8 + j] = Val(const=float(IV[j]))
                        for grp in (G_SCHEDULE[:4], G_SCHEDULE[4:]):
                            for li, (a, b, c, d, gx, gy) in enumerate(grp):
                                self.begin_lane(block_lanes[a], a)
                                self.G(vals, a, b, c, d, gx, gy)
                                self.end_lane()
                        last = rnd == ROUNDS - 1
                        new_state = [None] * 8
                        for j in range(8):
                            self.begin_lane(block_lanes[j % 4], j % 4)
                            if last:
                                dst = (lambda jj=j: out_tile[:][:, jj::8])
                                new_state[j] = self.soft_xor(
                                    None, vals[j], vals[8 + j], dst_ap=dst,
                                    materialize=True)
                                if new_state[j].is_const:
                                    cv = float(np.float32(new_state[j].const))
                                    self._run(lambda dd=dst, vv=cv:
                                              self.nc.vector.memset(dd(), vv))
                                    self.est["dve"] += ns_tt()
                            else:
                                new_state[j] = self.soft_xor(j, vals[j], vals[8 + j])
                            self.end_lane()
                        state = new_state
                    self.merge_lanes(block_lanes)
                    out_ap = self.out[r0:r0 + BLOCK_ROWS, :].rearrange("(p f) w -> p (f w)", p=P)
                    nc.sync.dma_start(out=out_ap, in_=out_tile[:])
        hoist_excess_waits(nc)
        return nc


def hoist_excess_waits(nc, max_waits=1):
    """Walrus can't encode >~2 sync waits per instruction; move excess into
    standalone NoOps (1 wait each) right before the instruction."""
    n_hoisted = 0
    for fu in nc.m.functions:
        for blk in fu.blocks:
            need = False
            for inst in blk.instructions:
                si = inst.sync_info
                if si is not None and len(si.on_wait) > max_waits:
                    need = True
                    break
            if not need:
                continue
            newl = []
            for inst in blk.instructions:
                si = inst.sync_info
                if si is not None and len(si.on_wait) > max_waits:
                    conds = list(si.on_wait)
                    keep = conds[-max_waits:]
                    for cnd in conds[:-max_waits]:
                        nop = mybir.InstNoOp(
                            name=nc.get_next_instruction_name(), ins=[], outs=[])
                        nop.engine = inst.engine
                        _bass_rust.wait_op(
                            nop, SemaphoreHandle(cnd.ant_name, cnd.id),
                            cnd.wait_value, "sem-ge", False)
                        newl.append(nop)
                        n_hoisted += 1
                    inst.sync_info = mybir.SyncInfo(on_wait=keep, on_update=list(si.on_update))
                newl.append(inst)
            blk.instructions = newl
    return n_hoisted


def build_program():
    p = Prog()
    nc = p.build()
    return nc, p


# ----------------------------------------------------------------- entry
_cache = {}


def _get_nc():
    if "nc" not in _cache:
        _cache["nc"] = build_program()[0]
    return _cache["nc"]


def kernel(message, _trace=False):
    """Full (2000000, 16) f32 in -> (2000000, 8) f32 out, 8-core data parallel."""
    from concourse.bass_utils import run_bass_kernel_spmd
    msg = np.ascontiguousarray(np.asarray(message, dtype=np.float32))
    nc = _get_nc()
    pad = PAD_ROWS - msg.shape[0]
    msgp = np.concatenate([msg, np.zeros((pad, 16), np.float32)]) if pad > 0 else msg
    shards = msgp.reshape(N_CORES, CORE_ROWS, 16)
    in_maps = [{"message": shards[i]} for i in range(N_CORES)]
    kw = dict(trace=True) if _trace else {}
    res = run_bass_kernel_spmd(nc, in_maps, core_ids=list(range(N_CORES)), **kw)
    out = np.concatenate([res.results[i]["out"] for i in range(N_CORES)], axis=0)
    if _trace:
        _cache["last_result"] = res
    return out[: msg.shape[0]]


# revision 34
# speedup vs baseline: 5.4962x; 1.0018x over previous
"""Blake2 soft-cipher Bass kernel for Trainium2 (8 NeuronCores, data parallel).

v2: affine-deferred values + interval-tracked tiny-value elimination +
3-engine (DVE/ACT/Pool) load balancing.

Key numerical facts (all verified against f32 reference semantics):
- rot16/24/32 of any soft_xor output are exact scales by 2^-n (the wrapped
  fraction is identically zero because xor outputs are >= ~0.0132 > 2^-9).
- rot32-scaled values (<= 2^-32) are sub-half-ULP against every downstream
  addend (c-words >= 0.0131 => half-ulp >= 2^-31 > 2^-32), and shift sigmoid
  inputs by < 0.03 ulp: both uses collapse to "the tiny operand is invisible".
- sigmoid of near-const inputs (width of the sigma output interval < ~2e-6)
  is replaced by its midpoint constant; the xor against a constant xs
  factors into a quadratic in ys: 1 - (a1+b1*ys)(a2+b2*ys).
- rot63(x) = 2x - [x>=0.5] exactly, up to a dropped x*2^-63 term that only
  survives rounding when frac(2x) == 0 (measure-zero, magnitude 5e-20).
- soft_xor / soft_add results carry deferred affine (scale, bias): biases
  fold into ACT sigmoid bias constants and stt scalars for free.
"""
import sys
sys.path.insert(0, "/opt/trn_rl_repo")
import math
import numpy as np
from concourse import bass, mybir
from concourse.tile import TileContext
from concourse.bass_primitives_rust import SemaphoreHandle
from concourse.bass import _bass_rust

A = mybir.AluOpType
F = mybir.ActivationFunctionType
DT = mybir.dt.float32

# ---------------------------------------------------------------- geometry
P = 128
FD = 980
BLOCK_ROWS = P * FD
BLOCKS = 2
CORE_ROWS = BLOCK_ROWS * BLOCKS
N_CORES = 8
TOTAL_ROWS = 2_000_000
PAD_ROWS = CORE_ROWS * N_CORES

ROUNDS = 10
G_SCHEDULE = [
    (0, 4, 8, 12, 0, 1), (1, 5, 9, 13, 2, 3), (2, 6, 10, 14, 4, 5), (3, 7, 11, 15, 6, 7),
    (0, 5, 10, 15, 8, 9), (1, 6, 11, 12, 10, 11), (2, 7, 8, 13, 12, 13), (3, 4, 9, 14, 14, 15),
]
_IV_INTS = [7640891576956012808, 13503953896175478587, 4354685564936845355,
            11912009170470909681, 5840696475078001361, 11170449401992604703,
            2270897969802886507, 6620516959819538809]
IV = (np.asarray(_IV_INTS, dtype=np.float32) / np.float32(2.0**64)).astype(np.float32)

EPSB = 3e-6            # interval widening for hw sigmoid inexactness
DROP_ADD_TOL = 2.5e-7  # drop soft_add addends with |value| below this
XS_CONST_TOL = 3e-6    # sigma-output interval width below which xs is const

POOL_TT = True         # allow Pool engine for tensor_tensor add/sub/mult
POOL_HANDICAP = 1.1    # decision-time multiplier on Pool cost (Pool ops add
                       # ~0.9us latency to serial chains; offload only under
                       # real DVE pressure)
DROP_D16 = True        # drop the 1.5e-5 rot16 addend into c (kills the whole
                       # runtime d-lineage; error budget ~1e-4 vs 2e-2 gate)
DROP_D16_TOL = 2e-5


def configure(fd=980, blocks=2):
    global FD, BLOCK_ROWS, BLOCKS, CORE_ROWS, PAD_ROWS
    FD = fd
    BLOCKS = blocks
    BLOCK_ROWS = P * FD
    CORE_ROWS = BLOCK_ROWS * BLOCKS
    PAD_ROWS = CORE_ROWS * N_CORES


def f32(x):
    return np.float32(x)


def sig64(z):
    z = float(z)
    if z >= 0:
        return 1.0 / (1.0 + math.exp(-z))
    e = math.exp(z)
    return e / (1.0 + e)


def sigc(z):
    """f32-rounded sigmoid of f64 arg."""
    return float(np.float32(sig64(z)))


# cost estimates (per-op ns at current FD), calibrated against TimelineSim
# with synthetic back-to-back op streams (includes per-op sync overheads)
def ns_tt():
    return (FD + 58) / 0.96 + 134

def ns_ts():
    return (FD / 2 + 58) / 0.96 + 174

def ns_act():
    return (FD + 352) / 1.2

def ns_pool_tt():
    return (FD / 1.2) / 0.42 + 170


class Val:
    """true_value = scale * ap[...] + bias, or a build-time const.
    lo/hi bound the TRUE value."""
    __slots__ = ("const", "ap", "scale", "bias", "lo", "hi")

    def __init__(self, const=None, ap=None, scale=1.0, bias=0.0, lo=None, hi=None):
        self.const = const
        self.ap = ap
        self.scale = float(scale)
        self.bias = float(bias)
        if const is not None:
            self.lo = self.hi = float(const)
        else:
            assert lo is not None and hi is not None, "tensor Val needs bounds"
            self.lo = float(lo)
            self.hi = float(hi)

    @property
    def is_const(self):
        return self.const is not None

    def absmax(self):
        return max(abs(self.lo), abs(self.hi))


def sadd_const(a, b):
    """f32-faithful soft_add of two consts."""
    s = f32(f32(a) + f32(b))
    z = f32(f32(10.0) * f32(s - f32(1.0)))
    w = f32(sig64(float(z)))
    return float(f32(s - w))


def sxor_const(x, y):
    xs = f32(sig64(float(f32(f32(10.0) * f32(f32(x) - f32(0.5))))))
    ys = f32(sig64(float(f32(f32(10.0) * f32(f32(y) - f32(0.5))))))
    t1 = f32(xs * f32(f32(1.0) - ys))
    t2 = f32(f32(f32(1.0) - xs) * ys)
    r = f32(f32(t1 + t2) - f32(t1 * t2))
    return float(min(max(float(r), 0.0), 1.0))


def rot63_const(c):
    c = f32(c)
    m = f32(math.floor(float(c) * 2.0))  # floor(2c) for c in [0,1)
    sl = f32(f32(c * f32(2.0**65)) - f32(m * f32(2.0**64)))
    s = f32(f32(c * f32(2.0)) + sl)
    # mod(s, 2^64) with s < 2^64 is identity
    return float(f32(s / f32(2.0**64)))


def half_ulp_floor(x):
    """smallest half-ulp among f32 values >= x (x > 0)."""
    assert x > 0
    _, e = math.frexp(x)  # x = m * 2^e, m in [0.5, 1)
    return 2.0 ** (e - 25)


class Prog:
    def __init__(self):
        self.nc = bass.Bass("TRN2")
        self.est = {"dve": 0.0, "act": 0.0, "pool": 0.0}
        self._lane = None
        self._lane_id = 0
        self._bias_tiles = {}
        self._bias_pool = None
        self._bias_count = 0

    # ---------------- lane machinery (same as baseline)
    def _run(self, fn):
        if self._lane is not None:
            self._lane.append(fn)
        else:
            fn()

    def begin_lane(self, lane, lane_id=0):
        self._lane = lane
        self._lane_id = lane_id

    def end_lane(self):
        self._lane = None

    def merge_lanes(self, lanes):
        lanes = [list(l) for l in lanes if l]
        while lanes:
            nxt = []
            for l in lanes:
                l.pop(0)()
                if l:
                    nxt.append(l)
            lanes = nxt

    # ---------------- scratch
    def scr(self):
        cell = {}
        tag = f"scr{self._lane_id or 0}"
        def get():
            if "t" not in cell:
                cell["t"] = self.scr_pool.tile([P, FD], DT, tag=tag, name=tag, bufs=3)
            return cell["t"][:]
        return get

    # ---------------- bias const tiles for ACT sigmoid
    def bias_ap(self, value):
        v = float(np.float32(value))
        if v not in self._bias_tiles:
            t = self._bias_pool.tile([P, 1], DT, tag=f"bias{self._bias_count}",
                                     name=f"bias{self._bias_count}")
            self._bias_count += 1
            # eager emit (before any lane-deferred consumer is flushed)
            self.nc.vector.memset(t[:], v)
            self.est["dve"] += 65.0
            self._bias_tiles[v] = t
        t = self._bias_tiles[v]
        return lambda: t[:]

    # ---------------- balanced emitters
    def tt(self, out, a, b, op):
        """tensor_tensor; Pool-eligible for add/sub/mult."""
        pool_ok = POOL_TT and op in (A.add, A.subtract, A.mult)
        cd, cp = ns_tt(), ns_pool_tt()
        if pool_ok and self.est["pool"] + cp < POOL_HANDICAP * (self.est["dve"] + cd):
            self.est["pool"] += cp
            self._run(lambda: self.nc.gpsimd.tensor_tensor(out(), a(), b(), op=op))
        else:
            self.est["dve"] += cd
            self._run(lambda: self.nc.vector.tensor_tensor(out(), a(), b(), op=op))

    def stt(self, out, in0, scalar, in1, op0, op1, rev0=False):
        def fn():
            i = self.nc.vector.scalar_tensor_tensor(
                out(), in0(), float(scalar), in1(), op0=op0, op1=op1)
            if rev0:
                i.ins.reverse0 = True
        self.est["dve"] += ns_tt()
        self._run(fn)

    def ts_cmp(self, out, in0, thresh, op0):
        """compare tensor_scalar — DVE only."""
        self.est["dve"] += ns_ts()
        self._run(lambda: self.nc.vector.tensor_scalar(out(), in0(), float(thresh), None, op0=op0))

    def affine(self, out, in0, scale, bias, strided=False):
        """out = scale*in0 + bias on DVE-ts or ACT-copy, balancer's choice."""
        scale = float(np.float32(scale))
        bias = float(np.float32(bias))
        cd = ns_tt() if strided else ns_ts()
        ca = ns_act()
        if self.est["act"] + ca < self.est["dve"] + cd:
            self.est["act"] += ca
            self._run(lambda: self.nc.scalar.activation(out(), in0(), F.Copy,
                                                        bias=bias, scale=scale))
        else:
            self.est["dve"] += cd
            if bias == 0.0:
                self._run(lambda: self.nc.vector.tensor_scalar(out(), in0(), scale, None, op0=A.mult))
            else:
                self._run(lambda: self.nc.vector.tensor_scalar(out(), in0(), scale, bias,
                                                               op0=A.mult, op1=A.add))

    def affine1m(self, out, in0):
        """out = 1 - in0."""
        cd, ca = ns_ts(), ns_act()
        if self.est["act"] + ca < self.est["dve"] + cd:
            self.est["act"] += ca
            self._run(lambda: self.nc.scalar.activation(out(), in0(), F.Copy,
                                                        bias=1.0, scale=-1.0))
        else:
            self.est["dve"] += cd
            def fn():
                i = self.nc.vector.tensor_scalar(out(), in0(), 1.0, None, op0=A.subtract)
                i.ins.reverse0 = True
            self._run(fn)

    def sigmoid(self, out, in_ap, scale, biasval):
        """out = sigmoid(scale * in + biasval); biasval via const tile."""
        b = self.bias_ap(biasval)
        s = float(np.float32(scale))
        self.est["act"] += ns_act()
        self._run(lambda: self.nc.scalar.activation(out(), in_ap(), F.Sigmoid,
                                                    bias=b(), scale=s))

    # ---------------- interval helpers
    def sig_interval(self, V):
        """interval of sigmoid(10*(V-0.5)) over V's bounds (widened)."""
        if V.is_const:
            x = sigc(f32(f32(10.0) * f32(f32(V.const) - f32(0.5))))
            return (x, x)
        lo = sig64(10.0 * (V.lo - EPSB - 0.5)) - 1e-7
        hi = sig64(10.0 * (V.hi + EPSB - 0.5)) + 1e-7
        return (lo, hi)

    # ---------------- soft primitives
    def soft_add(self, dst_slot, Av, Bv, dst_ap=None):
        if Av.is_const and Bv.is_const:
            return Val(const=sadd_const(Av.const, Bv.const))

        # skip/drop tiny addend (exactness or tolerance based)
        for X, Y in ((Av, Bv), (Bv, Av)):
            if Y.is_const or Y.absmax() > 2e-5:
                continue
            ymax = Y.absmax()
            exact_ok = (X.lo > 1e-30) and (ymax < 0.99 * half_ulp_floor(X.lo))
            drop_tol = DROP_D16_TOL if DROP_D16 else DROP_ADD_TOL
            if ymax <= drop_tol or exact_ok:
                if X.is_const:
                    return Val(const=sadd_const(X.const, 0.0))
                return self._sadd_finish(dst_slot, X.ap, X.scale, X.bias,
                                         Av, Bv, dst_ap)

        if Av.is_const or Bv.is_const:
            c, T = (Av.const, Bv) if Av.is_const else (Bv.const, Av)
            return self._sadd_finish(dst_slot, T.ap, T.scale, T.bias + c,
                                     Av, Bv, dst_ap)

        # combine two tensors
        sa, sb = Av.scale, Bv.scale
        h = self.scr()
        if sa == sb:
            self.tt(h, Av.ap, Bv.ap, A.add)
            ss = sa
        elif sa == -sb:
            self.tt(h, Av.ap, Bv.ap, A.subtract)
            ss = sa
        else:
            # keep |ratio| <= 1: fold the smaller-scale operand in scaled form
            if abs(sa) <= abs(sb):
                self.stt(h, Av.ap, sa / sb, Bv.ap, A.mult, A.add)
                ss = sb
            else:
                self.stt(h, Bv.ap, sb / sa, Av.ap, A.mult, A.add)
                ss = sa
        return self._sadd_finish(dst_slot, h, ss, Av.bias + Bv.bias, Av, Bv, dst_ap)

    def _sadd_finish(self, dst_slot, h, ss, beta, Av, Bv, dst_ap):
        # bounds of true output
        slo, shi = Av.lo + Bv.lo, Av.hi + Bv.hi
        lo = slo - sig64(10.0 * (shi - 1.0)) - EPSB
        hi = shi - sig64(10.0 * (slo - 1.0)) + EPSB
        w = self.scr()
        self.sigmoid(w, h, 10.0 * ss, 10.0 * beta - 10.0)
        dst = dst_ap if dst_ap is not None else self.scr()
        if ss == 1.0:
            self.tt(dst, h, w, A.subtract)
        else:
            self.stt(dst, h, ss, w, A.mult, A.subtract)
        return Val(ap=dst, scale=1.0, bias=beta, lo=lo, hi=hi)

    def soft_xor(self, dst_slot, Xv, Yv, dst_ap=None, materialize=False):
        if Xv.is_const and Yv.is_const:
            return Val(const=sxor_const(Xv.const, Yv.const))

        xi = self.sig_interval(Xv)
        yi = self.sig_interval(Yv)
        x_constish = Xv.is_const or (xi[1] - xi[0] <= XS_CONST_TOL)
        y_constish = Yv.is_const or (yi[1] - yi[0] <= XS_CONST_TOL)

        if x_constish and y_constish:
            # both sides' sigmoids constant: output is a build-time const
            xbar, ybar = (xi[0] + xi[1]) / 2, (yi[0] + yi[1]) / 2
            t1 = xbar * (1.0 - ybar)
            t2 = (1.0 - xbar) * ybar
            return Val(const=float(f32(t1 + t2 - t1 * t2)))
        if x_constish:
            return self._sxor_const_side(dst_slot, xi, Yv, yi, dst_ap, materialize)
        if y_constish:
            return self._sxor_const_side(dst_slot, yi, Xv, xi, dst_ap, materialize)
        return self._sxor_full(dst_slot, Xv, xi, Yv, yi, dst_ap, materialize)

    def _xor_bounds(self, xi, yi):
        corners = [(a, b) for a in xi for b in yi]
        vals = [a + b - 2 * a * b for a, b in corners]
        m, M = min(vals), max(vals)
        lo = max(0.0, m - m * m / 4.0) - EPSB
        hi = min(M, 1.0) + EPSB
        return lo, hi

    def _sxor_const_side(self, dst_slot, xi, Yv, yi, dst_ap, materialize):
        """xs is effectively const xbar; out = 1 - (a1+b1*ys)(a2+b2*ys)."""
        # canonicalize tiny-class midpoints so emitted constants (and ACT
        # bias tiles) repeat across rounds instead of tracking each
        # instance's slightly-different interval
        if xi[1] - xi[0] > 0:
            if xi[1] <= sig64(10 * (3e-10 - 0.5)) + 1e-6:
                xbar = sigc(10.0 * (2.0**-33 - 0.5))
            elif xi[1] <= sig64(10 * (3e-7 - 0.5)) + 1e-6:
                xbar = sigc(10.0 * (2.0**-25 - 0.5))
            elif xi[1] <= sig64(10 * (3e-5 - 0.5)) + 1e-6:
                xbar = sigc(10.0 * (2.0**-17 - 0.5))
            else:
                xbar = (xi[0] + xi[1]) / 2
        else:
            xbar = (xi[0] + xi[1]) / 2
        a1, b1 = 1.0 - xbar, xbar
        a2, b2 = 1.0, -(1.0 - xbar)
        c0 = a1 * a2
        c1 = a1 * b2 + b1 * a2
        c2 = b1 * b2
        lo, hi = self._xor_bounds(xi, yi)
        ys = self.scr()
        self.sigmoid(ys, Yv.ap, 10.0 * Yv.scale, 10.0 * Yv.bias - 5.0)
        dst = dst_ap if dst_ap is not None else self.scr()
        # Square-ACT variant: q = c2*ys^2 + c1*ys = b^2 - (s*ys + bq)^2 with
        # s = sqrt(-c2), bq = -c1/(2s); out = (1-c0-bq^2) + S. Pure-ACT (2 ops)
        # vs sigma + affine + tt. Pick by projected engine load.
        sq_s = math.sqrt(-c2)
        sq_b = -c1 / (2.0 * sq_s)
        ca, cd_extra = ns_act(), ns_ts() + ns_tt()
        use_sq = (not materialize) and (
            self.est["act"] + ca <= min(self.est["dve"] + cd_extra,
                                        self.est["pool"] + ns_pool_tt() + ns_ts()))
        if use_sq:
            b_ap = self.bias_ap(sq_b)
            self.est["act"] += ca
            self._run(lambda: self.nc.scalar.activation(
                dst(), ys(), F.Square, bias=b_ap(), scale=float(np.float32(sq_s))))
            return Val(ap=dst, scale=1.0, bias=1.0 - c0 - sq_b * sq_b, lo=lo, hi=hi)
        t = self.scr()
        self.affine(t, ys, c2, c1)
        if materialize:
            q = self.scr()
            self.tt(q, t, ys, A.mult)
            self.affine(dst, q, -1.0, 1.0 - c0, strided=True)
            return Val(ap=dst, scale=1.0, bias=0.0, lo=lo, hi=hi)
        self.tt(dst, t, ys, A.mult)
        return Val(ap=dst, scale=-1.0, bias=1.0 - c0, lo=lo, hi=hi)

    def _sxor_full(self, dst_slot, Xv, xi, Yv, yi, dst_ap, materialize):
        lo, hi = self._xor_bounds(xi, yi)
        xs = self.scr()
        ys = self.scr()
        t1 = self.scr()
        self.sigmoid(xs, Xv.ap, 10.0 * Xv.scale, 10.0 * Xv.bias - 5.0)
        self.sigmoid(ys, Yv.ap, 10.0 * Yv.scale, 10.0 * Yv.bias - 5.0)
        self.stt(t1, ys, 1.0, xs, A.subtract, A.mult, rev0=True)   # (1-ys)*xs
        self.stt(xs, xs, 1.0, ys, A.subtract, A.mult, rev0=True)   # xs <- t2=(1-xs)*ys
        self.affine1m(ys, xs)                                      # ys <- 1-t2
        dst = dst_ap if dst_ap is not None else self.scr()
        if materialize:
            self.stt(t1, t1, 1.0, ys, A.subtract, A.mult, rev0=True)  # (1-t1)(1-t2)
            self.affine(dst, t1, -1.0, 1.0, strided=True)
            return Val(ap=dst, scale=1.0, bias=0.0, lo=lo, hi=hi)
        self.stt(dst, t1, 1.0, ys, A.subtract, A.mult, rev0=True)
        return Val(ap=dst, scale=-1.0, bias=1.0, lo=lo, hi=hi)

    def soft_xor_dead(self, Xv, Yv):
        """soft_xor whose RUNTIME value is never consumed (only its bounds
        feed later const-folds). Emits nothing; returns a phantom Val whose
        ap raises if ever dereferenced."""
        if Xv.is_const and Yv.is_const:
            return Val(const=sxor_const(Xv.const, Yv.const))
        xi = self.sig_interval(Xv)
        yi = self.sig_interval(Yv)
        lo, hi = self._xor_bounds(xi, yi)
        def phantom():
            raise AssertionError("phantom (value-dead) soft_xor output was dereferenced")
        return Val(ap=phantom, scale=-1.0, bias=1.0, lo=lo, hi=hi)

    def rotate(self, slot, n, V):
        if V.is_const:
            if n in (16, 24, 32):
                return Val(const=float(f32(V.const)) * 2.0 ** (-n))
            assert n == 63
            return Val(const=rot63_const(V.const))
        if n in (16, 24, 32):
            need = {16: 2.0**-25, 24: 2.0**-17, 32: 2.0**-9}[n]
            assert V.lo >= need, f"rot{n} scale-defer needs lo>={need}, got {V.lo}"
            k = 2.0 ** (-n)
            return Val(ap=V.ap, scale=V.scale * k, bias=V.bias * k,
                       lo=V.lo * k, hi=V.hi * k)
        assert n == 63
        m = self.scr()
        dst = self.scr()
        s, b = V.scale, V.bias
        # mask = [X >= 0.5] with X = s*u + b
        if s < 0:
            self.ts_cmp(m, V.ap, (b - 0.5) / (-s), A.is_le)
        else:
            self.ts_cmp(m, V.ap, (0.5 - b) / s, A.is_ge)
        # r = 2X - m = (2s)*u - m, bias 2b deferred
        self.stt(dst, V.ap, 2.0 * s, m, A.mult, A.subtract)
        return Val(ap=dst, scale=1.0, bias=2.0 * b, lo=-EPSB, hi=1.0 + EPSB)

    # ---------------- G function
    def G(self, vals, a, b, c, d, xi, yi):
        mx = Val(ap=self.m_aps[xi], lo=0.0, hi=1.0)
        my = Val(ap=self.m_aps[yi], lo=0.0, hi=1.0)
        vals[a] = self.soft_add(a, vals[a], vals[b])
        vals[a] = self.soft_add(a, vals[a], mx)
        # #3's output only survives rot32 (sub-half-ULP everywhere) -> its
        # runtime value is dead; bounds still feed #5's skip and #10's consts
        vals[d] = self.soft_xor_dead(vals[d], vals[a])
        vals[d] = self.rotate(d, 32, vals[d])
        vals[c] = self.soft_add(c, vals[c], vals[d])
        # #6's output only survives rot24: dropped by #8 (<=2.5e-7) and
        # const-folded by #13 -> value-dead as well
        vals[b] = self.soft_xor_dead(vals[b], vals[c])
        vals[b] = self.rotate(b, 24, vals[b])
        vals[a] = self.soft_add(a, vals[a], vals[b])
        vals[a] = self.soft_add(a, vals[a], my)
        if DROP_D16:
            # with the d16 addend dropped in #12, #10's runtime value is
            # dead as well (its other consumers const-fold it)
            vals[d] = self.soft_xor_dead(vals[d], vals[a])
        else:
            vals[d] = self.soft_xor(d, vals[d], vals[a])
        vals[d] = self.rotate(d, 16, vals[d])
        vals[c] = self.soft_add(c, vals[c], vals[d])
        vals[b] = self.soft_xor(b, vals[b], vals[c])
        vals[b] = self.rotate(b, 63, vals[b])

    # ---------------- whole program
    def build(self, scr_bufs=10):
        nc = self.nc
        self.msg = nc.declare_dram_parameter("message", [CORE_ROWS, 16], DT, isOutput=False)
        self.out = nc.declare_dram_parameter("out", [CORE_ROWS, 8], DT, isOutput=True)
        with TileContext(nc) as tc:
            with (
                tc.tile_pool(name="persist", bufs=1) as pp,
                tc.tile_pool(name="scrp", bufs=scr_bufs) as sp,
            ):
                self.scr_pool = sp
                self._bias_pool = pp
                m_tiles = [pp.tile([P, 16 * FD], DT, tag=f"m_stage{i}", name=f"m_stage{i}")
                           for i in range(2)]
                out_tile = pp.tile([P, 8 * FD], DT, tag="out_stage", name="out_stage")
                self.v_aps = [None] * 16

                for blk in range(BLOCKS):
                    r0 = blk * BLOCK_ROWS
                    m_tile = m_tiles[blk % 2]
                    self.m_aps = [(lambda jj=j, mt=m_tile: mt[:][:, jj::16])
                                  for j in range(16)]
                    in_ap = self.msg[r0:r0 + BLOCK_ROWS, :].rearrange("(p f) w -> p (f w)", p=P)
                    nc.sync.dma_start(out=m_tile[:], in_=in_ap)
                    state = [Val(const=float(IV[j])) for j in range(8)]
                    # With b/c/d-words all const, the four a-word chains are
                    # fully independent across ALL rounds: emit each word's
                    # whole-block chain into one mega-lane (max scheduler slack)
                    block_lanes = [[] for _ in range(4)]
                    for rnd in range(ROUNDS):
                        vals = {}
                        for j in range(8):
                            vals[j] = state[j]
                            vals[8 + j] = Val(const=float(IV[j]))
                        for grp in (G_SCHEDULE[:4], G_SCHEDULE[4:]):
                            for li, (a, b, c, d, gx, gy) in enumerate(grp):
                                self.begin_lane(block_lanes[a], a)
                                self.G(vals, a, b, c, d, gx, gy)
                                self.end_lane()
                        last = rnd == ROUNDS - 1
                        new_state = [None] * 8
                        for j in range(8):
                            self.begin_lane(block_lanes[j % 4], j % 4)
                            if last:
                                dst = (lambda jj=j: out_tile[:][:, jj::8])
                                new_state[j] = self.soft_xor(
                                    None, vals[j], vals[8 + j], dst_ap=dst,
                                    materialize=True)
                                if new_state[j].is_const:
                                    cv = float(np.float32(new_state[j].const))
                                    self._run(lambda dd=dst, vv=cv:
                                              self.nc.vector.memset(dd(), vv))
                                    self.est["dve"] += ns_tt()
                            else:
                                new_state[j] = self.soft_xor(j, vals[j], vals[8 + j])
                            self.end_lane()
                        state = new_state
                    self.merge_lanes(block_lanes)
                    out_ap = self.out[r0:r0 + BLOCK_ROWS, :].rearrange("(p f) w -> p (f w)", p=P)
                    nc.sync.dma_start(out=out_ap, in_=out_tile[:])
        hoist_excess_waits(nc)
        return nc


def hoist_excess_waits(nc, max_waits=1):
    """Walrus can't encode >~2 sync waits per instruction; move excess into
    standalone NoOps (1 wait each) right before the instruction."""
    n_hoisted = 0
    for fu in nc.m.functions:
        for blk in fu.blocks:
            need = False
            for inst in blk.instructions:
                si = inst.sync_info
                if si is not None and len(si.on_wait) > max_waits:
                    need = True
                    break
            if not need:
                continue
            newl = []
            for inst in blk.instructions:
                si = inst.sync_info
                if si is not None and len(si.on_wait) > max_waits:
                    conds = list(si.on_wait)
                    keep = conds[-max_waits:]
                    for cnd in conds[:-max_waits]:
                        nop = mybir.InstNoOp(
                            name=nc.get_next_instruction_name(), ins=[], outs=[])
                        nop.engine = inst.engine
                        _bass_rust.wait_op(
                            nop, SemaphoreHandle(cnd.ant_name, cnd.id),
                            cnd.wait_value, "sem-ge", False)
                        newl.append(nop)
                        n_hoisted += 1
                    inst.sync_info = mybir.SyncInfo(on_wait=keep, on_update=list(si.on_update))
                newl.append(inst)
            blk.instructions = newl
    return n_hoisted


def build_program():
    p = Prog()
    nc = p.build()
    return nc, p


# ----------------------------------------------------------------- entry
_cache = {}


def _get_nc():
    if "nc" not in _cache:
        _cache["nc"] = build_program()[0]
    return _cache["nc"]


def kernel(message, _trace=False):
    """Full (2000000, 16) f32 in -> (2000000, 8) f32 out, 8-core data parallel."""
    from concourse.bass_utils import run_bass_kernel_spmd
    msg = np.ascontiguousarray(np.asarray(message, dtype=np.float32))
    nc = _get_nc()
    pad = PAD_ROWS - msg.shape[0]
    msgp = np.concatenate([msg, np.zeros((pad, 16), np.float32)]) if pad > 0 else msg
    shards = msgp.reshape(N_CORES, CORE_ROWS, 16)
    in_maps = [{"message": shards[i]} for i in range(N_CORES)]
    kw = dict(trace=True) if _trace else {}
    res = run_bass_kernel_spmd(nc, in_maps, core_ids=list(range(N_CORES)), **kw)
    out = np.concatenate([res.results[i]["out"] for i in range(N_CORES)], axis=0)
    if _trace:
        _cache["last_result"] = res
    return out[: msg.shape[0]]
